# revision 1
# baseline (speedup 1.0000x reference)
"""Trainium2 Bass kernel for nn_DataAugmentation (RandomResizedCrop + hflip batch).

Strategy
--------
Data parallel over batch: core k handles samples [8k, 8k+8).

All data-dependent work (RNG replication, crop params, bilinear weights,
hflip column remap) is tiny and happens on host; it is encoded into two
small per-sample interpolation matrices so the device does only dense
matmuls over its full input shard:

    out[b, c] = (A_yT[b].T @ img[b, c]) @ A_xT[b]
              = stage1 (row interp)      stage2 (col interp)

with img the ORIGINAL image (incl. metadata col, 512x513); the horizontal
flip is absorbed into A_xT's source-column index map.

Device stage 1 computes t.T = img.T @ A_yT chunk-wise (lhsT = img slice —
natural row-major layout, no transposes anywhere), stage 2 computes
out = (t.T).T @ A_xT.
"""

import numpy as np

SIZE = 224
HM = 512
WM = 513  # includes metadata column
B_FULL = 64
C = 3
N_CORES = 8
BPC = B_FULL // N_CORES  # samples per core
KY = HM // 128  # k-tiles over rows / cols

# dtype config: "float32" (exact), "bfloat16", "float32r"
DT_NAME = "float32"
NP = 224  # padded free dim (>=224; use 256 for float32r speed)

SCALE = (0.1, 1.0)
RATIO = (0.8, 1.25)
N_TRIES = 10

_prog_cache = {}


# ---------------------------------------------------------------- host math
def _compute_params(x):
    """Replicates reference._get_params + flip RNG exactly (jax on CPU)."""
    import jax
    import jax.numpy as jnp

    cpu = jax.devices("cpu")[0]
    with jax.default_device(cpu):
        B = x.shape[0]
        H = x[:, 0, 0, -1].astype(np.int32)
        W = x[:, 1, 0, -1].astype(np.int32)
        key = jax.random.key(42)
        kflip, kparams = jax.random.split(key)
        flip_mask = np.asarray(jax.random.uniform(kflip, (B,)) > 0.5)

        Hf = jnp.asarray(H, jnp.float32)
        Wf = jnp.asarray(W, jnp.float32)
        area = Hf * Wf
        log_lo = np.log(RATIO[0]).astype(np.float32)
        log_hi = np.log(RATIO[1]).astype(np.float32)
        final_h = jnp.full((B,), -1.0, dtype=jnp.float32)
        final_w = jnp.full((B,), -1.0, dtype=jnp.float32)
        final_i = jnp.full((B,), -1.0, dtype=jnp.float32)
        final_j = jnp.full((B,), -1.0, dtype=jnp.float32)
        success = jnp.zeros((B,), dtype=bool)
        for t in range(N_TRIES):
            k1, k2, k3, k4 = jax.random.split(jax.random.fold_in(kparams, t), 4)
            target_area = area * jax.random.uniform(
                k1, (B,), minval=SCALE[0], maxval=SCALE[1]
            )
            aspect = jnp.exp(jax.random.uniform(k2, (B,), minval=log_lo, maxval=log_hi))
            crop_w = jnp.round(jnp.sqrt(target_area * aspect))
            crop_h = jnp.round(jnp.sqrt(target_area / aspect))
            valid = (
                (crop_w > 0) & (crop_w <= Wf) & (crop_h > 0) & (crop_h <= Hf) & (~success)
            )
            max_i = jnp.clip(Hf - crop_h + 1.0, 1.0, None)
            max_j = jnp.clip(Wf - crop_w + 1.0, 1.0, None)
            rand_i = jnp.floor(jax.random.uniform(k3, (B,)) * max_i)
            rand_j = jnp.floor(jax.random.uniform(k4, (B,)) * max_j)
            final_h = jnp.where(valid, crop_h, final_h)
            final_w = jnp.where(valid, crop_w, final_w)
            final_i = jnp.where(valid, rand_i, final_i)
            final_j = jnp.where(valid, rand_j, final_j)
            success = success | valid
        failed = ~success
        in_ratio = Wf / Hf
        fb_w = jnp.where(in_ratio > RATIO[1], jnp.round(Hf * RATIO[1]), Wf)
        fb_h = jnp.where(in_ratio < RATIO[0], jnp.round(Wf / RATIO[0]), Hf)
        fb_i = jnp.floor((Hf - fb_h) / 2.0)
        fb_j = jnp.floor((Wf - fb_w) / 2.0)
        final_h = jnp.where(failed, fb_h, final_h)
        final_w = jnp.where(failed, fb_w, final_w)
        final_i = jnp.where(failed, fb_i, final_i)
        final_j = jnp.where(failed, fb_j, final_j)
        i = np.asarray(final_i.astype(jnp.int32))
        j = np.asarray(final_j.astype(jnp.int32))
        h = np.asarray(final_h.astype(jnp.int32))
        w = np.asarray(final_w.astype(jnp.int32))
    return flip_mask, i, j, h, w


def _axis_weights(start, length, n_src_max):
    lf = np.float32(length)
    s = (np.arange(SIZE, dtype=np.float32) + np.float32(0.5)) * lf / np.float32(
        SIZE
    ) - np.float32(0.5)
    s = np.clip(s, np.float32(0.0), lf - np.float32(1.0))
    s0 = np.floor(s)
    frac = (s - s0).astype(np.float32)
    i0 = np.clip(s0.astype(np.int64) + start, 0, n_src_max - 1)
    hi = int(np.clip(start + length - 1, 0, n_src_max - 1))
    i1 = np.clip(i0 + 1, 0, hi)
    return i0, i1, frac


def _build_weights(x):
    """A_yT [B, 512, NP], A_xT [B, 513, NP] f32 (flip folded into A_xT)."""
    Bn = x.shape[0]
    flip_mask, i, j, h, w = _compute_params(x)
    ayt = np.zeros((Bn, HM, NP), dtype=np.float32)
    axt = np.zeros((Bn, WM, NP), dtype=np.float32)
    cols = np.arange(SIZE)
    for b in range(Bn):
        y0, y1, wy = _axis_weights(int(i[b]), int(h[b]), HM)
        np.add.at(ayt[b], (y0, cols), (1.0 - wy))
        np.add.at(ayt[b], (y1, cols), wy)
        x0, x1, wx = _axis_weights(int(j[b]), int(w[b]), HM)
        if flip_mask[b]:
            x0 = HM - x0
            x1 = HM - x1
        np.add.at(axt[b], (x0, cols), (1.0 - wx))
        np.add.at(axt[b], (x1, cols), wx)
    return ayt, axt


# ---------------------------------------------------------------- device prog
def _build_program(use_meta):
    import concourse.mybir as mybir
    import concourse.tile as tile
    from concourse import bacc

    DT = getattr(mybir.dt, DT_NAME)
    F32 = mybir.dt.float32

    nc = bacc.Bacc(None)
    x_d = nc.dram_tensor("x_s", [BPC, C, HM, WM], DT, kind="ExternalInput")
    ayt_d = nc.dram_tensor("ayt", [BPC, HM, NP], DT, kind="ExternalInput")
    axt_d = nc.dram_tensor("axt", [BPC, WM, NP], DT, kind="ExternalInput")
    out_d = nc.dram_tensor("out", [BPC, C, SIZE, SIZE], F32, kind="ExternalOutput")

    with tile.TileContext(nc) as tc:
        with (
            tc.tile_pool(name="img", bufs=3) as img_pool,
            tc.tile_pool(name="wy", bufs=2) as wy_pool,
            tc.tile_pool(name="wx", bufs=2) as wx_pool,
            tc.tile_pool(name="tq", bufs=12) as tq_pool,
            tc.tile_pool(name="ob", bufs=4) as out_pool,
            tc.tile_pool(name="ps1", bufs=3, space="PSUM") as ps1,
            tc.tile_pool(name="ps2", bufs=3, space="PSUM") as ps2,
        ):

            def emit_stage2(job):
                b, c, tqs, tmeta, axt_t, axt_m = job
                for m2 in range(2):
                    m2sz = min(128, SIZE - m2 * 128)
                    sl = slice(m2 * 128, m2 * 128 + m2sz)
                    po = ps2.tile([128, NP], F32, tag="ps2")
                    for k2 in range(KY):
                        nc.tensor.matmul(
                            po[:m2sz, :],
                            tqs[k2][:, sl],
                            axt_t[:, k2, :],
                            start=(k2 == 0),
                            stop=(k2 == KY - 1 and not use_meta),
                        )
                    if use_meta:
                        nc.tensor.matmul(
                            po[:m2sz, :],
                            tmeta[:1, sl],
                            axt_m[:1, :],
                            start=False,
                            stop=True,
                        )
                    ob = out_pool.tile([128, SIZE], F32, tag="ob")
                    nc.vector.tensor_copy(ob[:m2sz, :], po[:m2sz, :SIZE])
                    nc.sync.dma_start(out_d[b, c, sl, :], ob[:m2sz, :])

            prev = None
            for b in range(BPC):
                ayt_t = wy_pool.tile([128, KY, NP], DT, tag="wy")
                nc.sync.dma_start(
                    ayt_t[:], ayt_d[b].rearrange("(a p) n -> p a n", p=128)
                )
                axt_t = wx_pool.tile([128, KY, NP], DT, tag="wx")
                nc.sync.dma_start(
                    axt_t[:], axt_d[b, :HM].rearrange("(a p) n -> p a n", p=128)
                )
                axt_m = None
                if use_meta:
                    axt_m = wx_pool.tile([1, NP], DT, tag="wxm")
                    nc.sync.dma_start(axt_m[:], axt_d[b, HM : HM + 1, :])
                for c in range(C):
                    img_t = img_pool.tile([128, KY, WM], DT, tag="img")
                    nc.sync.dma_start(
                        img_t[:], x_d[b, c].rearrange("(a p) w -> p a w", p=128)
                    )
                    tqs = []
                    for q in range(KY):
                        p1 = ps1.tile([128, NP], F32, tag="ps1")
                        for k in range(KY):
                            nc.tensor.matmul(
                                p1[:, :],
                                img_t[:, k, q * 128 : (q + 1) * 128],
                                ayt_t[:, k, :],
                                start=(k == 0),
                                stop=(k == KY - 1),
                            )
                        tq = tq_pool.tile([128, SIZE], DT, tag="tq")
                        nc.vector.tensor_copy(tq[:], p1[:, :SIZE])
                        tqs.append(tq)
                    tmeta = None
                    if use_meta:
                        pm = ps1.tile([128, NP], F32, tag="ps1")
                        for k in range(KY):
                            nc.tensor.matmul(
                                pm[:1, :],
                                img_t[:, k, HM : HM + 1],
                                ayt_t[:, k, :],
                                start=(k == 0),
                                stop=(k == KY - 1),
                            )
                        tmeta = tq_pool.tile([1, SIZE], DT, tag="tqm")
                        nc.vector.tensor_copy(tmeta[:], pm[:1, :SIZE])
                    if prev is not None:
                        emit_stage2(prev)
                    prev = (b, c, tqs, tmeta, axt_t, axt_m)
            emit_stage2(prev)

    nc.compile()
    return nc


def _get_program(use_meta):
    key = (use_meta, DT_NAME, NP)
    if key not in _prog_cache:
        _prog_cache[key] = _build_program(use_meta)
    return _prog_cache[key]


def _np_dt(name):
    if name == "bfloat16":
        import ml_dtypes

        return ml_dtypes.bfloat16
    return np.float32


# ---------------------------------------------------------------- entry point
def kernel(x, trace=False):
    from concourse.bass_utils import run_bass_kernel_spmd

    x = np.ascontiguousarray(np.asarray(x, dtype=np.float32))
    assert x.shape == (B_FULL, C, HM, WM), x.shape

    ayt, axt = _build_weights(x)
    use_meta = bool(np.abs(axt[:, HM, :]).max() > 0)

    nc = _get_program(use_meta)

    ndt = _np_dt(DT_NAME)
    xc = x.astype(ndt, copy=False)
    aytc = ayt.astype(ndt, copy=False)
    axtc = axt.astype(ndt, copy=False)

    in_maps = []
    for k in range(N_CORES):
        sl = slice(k * BPC, (k + 1) * BPC)
        in_maps.append(
            {
                "x_s": np.ascontiguousarray(xc[sl]),
                "ayt": np.ascontiguousarray(aytc[sl]),
                "axt": np.ascontiguousarray(axtc[sl]),
            }
        )

    res = run_bass_kernel_spmd(nc, in_maps, list(range(N_CORES)), trace=trace)
    out = np.concatenate([res.results[k]["out"] for k in range(N_CORES)], axis=0)
    if trace:
        kernel.last_exec_ns = res.exec_time_ns
        kernel.last_results = res
    return out


# revision 5
# speedup vs baseline: 1.4847x; 1.4847x over previous
"""Trainium2 Bass kernel for nn_DataAugmentation (RandomResizedCrop + hflip batch).

Strategy
--------
Data parallel over batch: core k handles samples [8k, 8k+8).

All data-dependent work (RNG replication, crop params, bilinear weights,
hflip column remap) is tiny and happens on host; it is encoded into two
small per-sample interpolation matrices so the device does only dense
matmuls over its full input shard:

    out[b, c] = (A_yT[b].T @ img[b, c]) @ A_xT[b]
              = stage1 (row interp)      stage2 (col interp)

with img the ORIGINAL image (incl. metadata col, 512x513); the horizontal
flip is absorbed into A_xT's source-column index map.

Device stage 1 computes t.T = img.T @ A_yT chunk-wise (lhsT = img slice —
natural row-major layout, no transposes anywhere), stage 2 computes
out = (t.T).T @ A_xT.
"""

import numpy as np

SIZE = 224
HM = 512
WM = 513  # includes metadata column
B_FULL = 64
C = 3
N_CORES = 8
BPC = B_FULL // N_CORES  # samples per core
KY = HM // 128  # k-tiles over rows / cols

# dtype config: "float32" (exact), "bfloat16", "float32r"
DT_NAME = "float32r"
NP = 256  # padded free dim (>=224; 256 gives float32r full-rate matmuls)
F32R_BITS = 11  # HW-probed: FP32R keeps 11 mantissa bits, round-nearest-even
F32R_MODE = "rne"

SCALE = (0.1, 1.0)
RATIO = (0.8, 1.25)
N_TRIES = 10

_prog_cache = {}


# ---------------------------------------------------------------- host math
def _compute_params(x):
    """Replicates reference._get_params + flip RNG exactly (jax on CPU)."""
    import jax
    import jax.numpy as jnp

    cpu = jax.devices("cpu")[0]
    with jax.default_device(cpu):
        B = x.shape[0]
        H = x[:, 0, 0, -1].astype(np.int32)
        W = x[:, 1, 0, -1].astype(np.int32)
        key = jax.random.key(42)
        kflip, kparams = jax.random.split(key)
        flip_mask = np.asarray(jax.random.uniform(kflip, (B,)) > 0.5)

        Hf = jnp.asarray(H, jnp.float32)
        Wf = jnp.asarray(W, jnp.float32)
        area = Hf * Wf
        log_lo = np.log(RATIO[0]).astype(np.float32)
        log_hi = np.log(RATIO[1]).astype(np.float32)
        final_h = jnp.full((B,), -1.0, dtype=jnp.float32)
        final_w = jnp.full((B,), -1.0, dtype=jnp.float32)
        final_i = jnp.full((B,), -1.0, dtype=jnp.float32)
        final_j = jnp.full((B,), -1.0, dtype=jnp.float32)
        success = jnp.zeros((B,), dtype=bool)
        for t in range(N_TRIES):
            k1, k2, k3, k4 = jax.random.split(jax.random.fold_in(kparams, t), 4)
            target_area = area * jax.random.uniform(
                k1, (B,), minval=SCALE[0], maxval=SCALE[1]
            )
            aspect = jnp.exp(jax.random.uniform(k2, (B,), minval=log_lo, maxval=log_hi))
            crop_w = jnp.round(jnp.sqrt(target_area * aspect))
            crop_h = jnp.round(jnp.sqrt(target_area / aspect))
            valid = (
                (crop_w > 0) & (crop_w <= Wf) & (crop_h > 0) & (crop_h <= Hf) & (~success)
            )
            max_i = jnp.clip(Hf - crop_h + 1.0, 1.0, None)
            max_j = jnp.clip(Wf - crop_w + 1.0, 1.0, None)
            rand_i = jnp.floor(jax.random.uniform(k3, (B,)) * max_i)
            rand_j = jnp.floor(jax.random.uniform(k4, (B,)) * max_j)
            final_h = jnp.where(valid, crop_h, final_h)
            final_w = jnp.where(valid, crop_w, final_w)
            final_i = jnp.where(valid, rand_i, final_i)
            final_j = jnp.where(valid, rand_j, final_j)
            success = success | valid
        failed = ~success
        in_ratio = Wf / Hf
        fb_w = jnp.where(in_ratio > RATIO[1], jnp.round(Hf * RATIO[1]), Wf)
        fb_h = jnp.where(in_ratio < RATIO[0], jnp.round(Wf / RATIO[0]), Hf)
        fb_i = jnp.floor((Hf - fb_h) / 2.0)
        fb_j = jnp.floor((Wf - fb_w) / 2.0)
        final_h = jnp.where(failed, fb_h, final_h)
        final_w = jnp.where(failed, fb_w, final_w)
        final_i = jnp.where(failed, fb_i, final_i)
        final_j = jnp.where(failed, fb_j, final_j)
        i = np.asarray(final_i.astype(jnp.int32))
        j = np.asarray(final_j.astype(jnp.int32))
        h = np.asarray(final_h.astype(jnp.int32))
        w = np.asarray(final_w.astype(jnp.int32))
    return flip_mask, i, j, h, w


def _axis_weights(start, length, n_src_max):
    lf = np.float32(length)
    s = (np.arange(SIZE, dtype=np.float32) + np.float32(0.5)) * lf / np.float32(
        SIZE
    ) - np.float32(0.5)
    s = np.clip(s, np.float32(0.0), lf - np.float32(1.0))
    s0 = np.floor(s)
    frac = (s - s0).astype(np.float32)
    i0 = np.clip(s0.astype(np.int64) + start, 0, n_src_max - 1)
    hi = int(np.clip(start + length - 1, 0, n_src_max - 1))
    i1 = np.clip(i0 + 1, 0, hi)
    return i0, i1, frac


def _build_weights(x):
    """A_yT [B, 512, NP], A_xT [B, 513, NP] f32 (flip folded into A_xT)."""
    Bn = x.shape[0]
    flip_mask, i, j, h, w = _compute_params(x)
    ayt = np.zeros((Bn, HM, NP), dtype=np.float32)
    axt = np.zeros((Bn, WM, NP), dtype=np.float32)
    cols = np.arange(SIZE)
    for b in range(Bn):
        y0, y1, wy = _axis_weights(int(i[b]), int(h[b]), HM)
        np.add.at(ayt[b], (y0, cols), (1.0 - wy))
        np.add.at(ayt[b], (y1, cols), wy)
        x0, x1, wx = _axis_weights(int(j[b]), int(w[b]), HM)
        if flip_mask[b]:
            x0 = HM - x0
            x1 = HM - x1
        np.add.at(axt[b], (x0, cols), (1.0 - wx))
        np.add.at(axt[b], (x1, cols), wx)
    return ayt, axt


# ---------------------------------------------------------------- device prog
def _build_program(use_meta):
    import concourse.mybir as mybir
    import concourse.tile as tile
    from concourse import bacc

    DT = getattr(mybir.dt, DT_NAME)
    F32 = mybir.dt.float32

    nc = bacc.Bacc(None)
    x_d = nc.dram_tensor("x_s", [BPC, C, HM, WM], DT, kind="ExternalInput")
    ayt_d = nc.dram_tensor("ayt", [BPC, HM, NP], DT, kind="ExternalInput")
    axt_d = nc.dram_tensor("axt", [BPC, WM, NP], DT, kind="ExternalInput")
    out_d = nc.dram_tensor("out", [BPC, C, SIZE, SIZE], F32, kind="ExternalOutput")

    with tile.TileContext(nc) as tc:
        with (
            tc.tile_pool(name="img", bufs=3) as img_pool,
            tc.tile_pool(name="wy", bufs=2) as wy_pool,
            tc.tile_pool(name="wx", bufs=2) as wx_pool,
            tc.tile_pool(name="tq", bufs=12) as tq_pool,
            tc.tile_pool(name="ob", bufs=4) as out_pool,
            tc.tile_pool(name="ps1", bufs=3, space="PSUM") as ps1,
            tc.tile_pool(name="ps2", bufs=3, space="PSUM") as ps2,
        ):

            def emit_stage2(job):
                b, c, tqs, tmeta, axt_t, axt_m = job
                for m2 in range(2):
                    m2sz = min(128, SIZE - m2 * 128)
                    sl = slice(m2 * 128, m2 * 128 + m2sz)
                    po = ps2.tile([128, NP], F32, tag="ps2")
                    for k2 in range(KY):
                        nc.tensor.matmul(
                            po[:m2sz, :],
                            tqs[k2][:, sl],
                            axt_t[:, k2, :],
                            start=(k2 == 0),
                            stop=(k2 == KY - 1 and not use_meta),
                        )
                    if use_meta:
                        nc.tensor.matmul(
                            po[:m2sz, :],
                            tmeta[:1, sl],
                            axt_m[:1, :],
                            start=False,
                            stop=True,
                        )
                    ob = out_pool.tile([128, SIZE], F32, tag="ob")
                    nc.vector.tensor_copy(ob[:m2sz, :], po[:m2sz, :SIZE])
                    nc.sync.dma_start(out_d[b, c, sl, :], ob[:m2sz, :])

            prev = None
            for b in range(BPC):
                ayt_t = wy_pool.tile([128, KY, NP], DT, tag="wy")
                nc.sync.dma_start(
                    ayt_t[:], ayt_d[b].rearrange("(a p) n -> p a n", p=128)
                )
                axt_t = wx_pool.tile([128, KY, NP], DT, tag="wx")
                nc.sync.dma_start(
                    axt_t[:], axt_d[b, :HM].rearrange("(a p) n -> p a n", p=128)
                )
                axt_m = None
                if use_meta:
                    axt_m = wx_pool.tile([1, NP], DT, tag="wxm")
                    nc.sync.dma_start(axt_m[:], axt_d[b, HM : HM + 1, :])
                for c in range(C):
                    img_t = img_pool.tile([128, KY, WM], DT, tag="img")
                    nc.sync.dma_start(
                        img_t[:], x_d[b, c].rearrange("(a p) w -> p a w", p=128)
                    )
                    tqs = []
                    for q in range(KY):
                        p1 = ps1.tile([128, NP], F32, tag="ps1")
                        for k in range(KY):
                            nc.tensor.matmul(
                                p1[:, :],
                                img_t[:, k, q * 128 : (q + 1) * 128],
                                ayt_t[:, k, :],
                                start=(k == 0),
                                stop=(k == KY - 1),
                            )
                        tq = tq_pool.tile([128, SIZE], DT, tag="tq")
                        nc.vector.tensor_copy(tq[:], p1[:, :SIZE])
                        tqs.append(tq)
                    tmeta = None
                    if use_meta:
                        pm = ps1.tile([128, NP], F32, tag="ps1")
                        for k in range(KY):
                            nc.tensor.matmul(
                                pm[:1, :],
                                img_t[:, k, HM : HM + 1],
                                ayt_t[:, k, :],
                                start=(k == 0),
                                stop=(k == KY - 1),
                            )
                        tmeta = tq_pool.tile([1, SIZE], DT, tag="tqm")
                        nc.vector.tensor_copy(tmeta[:], pm[:1, :SIZE])
                    if prev is not None:
                        emit_stage2(prev)
                    prev = (b, c, tqs, tmeta, axt_t, axt_m)
            emit_stage2(prev)

    nc.compile()
    return nc


def _get_program(use_meta):
    key = (use_meta, DT_NAME, NP)
    if key not in _prog_cache:
        _prog_cache[key] = _build_program(use_meta)
    return _prog_cache[key]


def _np_dt(name):
    if name == "bfloat16":
        import ml_dtypes

        return ml_dtypes.bfloat16
    return np.float32


def _round_f32r(a):
    """Pre-round fp32 data to the FP32R grid the PE uses, so device inputs
    are already 'rounded to FP32r' and results are deterministic."""
    if F32R_BITS is None or F32R_BITS >= 23:
        return a
    drop = 23 - F32R_BITS
    u = np.ascontiguousarray(a).view(np.uint32)
    if F32R_MODE == "rne":
        half = np.uint32((1 << (drop - 1)) - 1)
        lsb = (u >> drop) & np.uint32(1)
        u = u + half + lsb
    u = (u >> drop) << drop
    return u.view(np.float32)


# ---------------------------------------------------------------- entry point
def kernel(x, trace=False):
    from concourse.bass_utils import run_bass_kernel_spmd

    x = np.ascontiguousarray(np.asarray(x, dtype=np.float32))
    assert x.shape == (B_FULL, C, HM, WM), x.shape

    ayt, axt = _build_weights(x)
    use_meta = bool(np.abs(axt[:, HM, :]).max() > 0)

    nc = _get_program(use_meta)

    ndt = _np_dt(DT_NAME)
    xc = x.astype(ndt, copy=False)
    aytc = ayt.astype(ndt, copy=False)
    axtc = axt.astype(ndt, copy=False)
    if DT_NAME == "float32r":
        xc = _round_f32r(xc)
        aytc = _round_f32r(aytc)
        axtc = _round_f32r(axtc)

    in_maps = []
    for k in range(N_CORES):
        sl = slice(k * BPC, (k + 1) * BPC)
        in_maps.append(
            {
                "x_s": np.ascontiguousarray(xc[sl]),
                "ayt": np.ascontiguousarray(aytc[sl]),
                "axt": np.ascontiguousarray(axtc[sl]),
            }
        )

    res = run_bass_kernel_spmd(nc, in_maps, list(range(N_CORES)), trace=trace)
    out = np.concatenate([res.results[k]["out"] for k in range(N_CORES)], axis=0)
    if trace:
        kernel.last_exec_ns = res.exec_time_ns
        kernel.last_results = res
    return out


# revision 7
# speedup vs baseline: 1.5278x; 1.0291x over previous
"""Trainium2 Bass kernel for nn_DataAugmentation (RandomResizedCrop + hflip batch).

Strategy
--------
Data parallel over batch: core k handles samples [8k, 8k+8).

All data-dependent work (RNG replication, crop params, bilinear weights,
hflip column remap) is tiny and happens on host; it is encoded into two
small per-sample interpolation matrices so the device does only dense
matmuls over its full input shard:

    out[b, c] = (A_yT[b].T @ img[b, c]) @ A_xT[b]
              = stage1 (row interp)      stage2 (col interp)

with img the ORIGINAL image (incl. metadata col, 512x513); the horizontal
flip is absorbed into A_xT's source-column index map.

Device stage 1 computes t.T = img.T @ A_yT chunk-wise (lhsT = img slice —
natural row-major layout, no transposes anywhere), stage 2 computes
out = (t.T).T @ A_xT.
"""

import numpy as np

SIZE = 224
HM = 512
WM = 513  # includes metadata column
B_FULL = 64
C = 3
N_CORES = 8
BPC = B_FULL // N_CORES  # samples per core
KY = HM // 128  # k-tiles over rows / cols

# dtype config: "float32" (exact), "bfloat16", "float32r"
DT_NAME = "float32r"
NP = 256  # padded free dim (>=224; 256 gives float32r full-rate matmuls)
F32R_BITS = 11  # HW-probed: FP32R keeps 11 mantissa bits, round-nearest-even
F32R_MODE = "rne"

SCALE = (0.1, 1.0)
RATIO = (0.8, 1.25)
N_TRIES = 10

_prog_cache = {}


# ---------------------------------------------------------------- host math
def _compute_params(x):
    """Replicates reference._get_params + flip RNG exactly (jax on CPU)."""
    import jax
    import jax.numpy as jnp

    cpu = jax.devices("cpu")[0]
    with jax.default_device(cpu):
        B = x.shape[0]
        H = x[:, 0, 0, -1].astype(np.int32)
        W = x[:, 1, 0, -1].astype(np.int32)
        key = jax.random.key(42)
        kflip, kparams = jax.random.split(key)
        flip_mask = np.asarray(jax.random.uniform(kflip, (B,)) > 0.5)

        Hf = jnp.asarray(H, jnp.float32)
        Wf = jnp.asarray(W, jnp.float32)
        area = Hf * Wf
        log_lo = np.log(RATIO[0]).astype(np.float32)
        log_hi = np.log(RATIO[1]).astype(np.float32)
        final_h = jnp.full((B,), -1.0, dtype=jnp.float32)
        final_w = jnp.full((B,), -1.0, dtype=jnp.float32)
        final_i = jnp.full((B,), -1.0, dtype=jnp.float32)
        final_j = jnp.full((B,), -1.0, dtype=jnp.float32)
        success = jnp.zeros((B,), dtype=bool)
        for t in range(N_TRIES):
            k1, k2, k3, k4 = jax.random.split(jax.random.fold_in(kparams, t), 4)
            target_area = area * jax.random.uniform(
                k1, (B,), minval=SCALE[0], maxval=SCALE[1]
            )
            aspect = jnp.exp(jax.random.uniform(k2, (B,), minval=log_lo, maxval=log_hi))
            crop_w = jnp.round(jnp.sqrt(target_area * aspect))
            crop_h = jnp.round(jnp.sqrt(target_area / aspect))
            valid = (
                (crop_w > 0) & (crop_w <= Wf) & (crop_h > 0) & (crop_h <= Hf) & (~success)
            )
            max_i = jnp.clip(Hf - crop_h + 1.0, 1.0, None)
            max_j = jnp.clip(Wf - crop_w + 1.0, 1.0, None)
            rand_i = jnp.floor(jax.random.uniform(k3, (B,)) * max_i)
            rand_j = jnp.floor(jax.random.uniform(k4, (B,)) * max_j)
            final_h = jnp.where(valid, crop_h, final_h)
            final_w = jnp.where(valid, crop_w, final_w)
            final_i = jnp.where(valid, rand_i, final_i)
            final_j = jnp.where(valid, rand_j, final_j)
            success = success | valid
        failed = ~success
        in_ratio = Wf / Hf
        fb_w = jnp.where(in_ratio > RATIO[1], jnp.round(Hf * RATIO[1]), Wf)
        fb_h = jnp.where(in_ratio < RATIO[0], jnp.round(Wf / RATIO[0]), Hf)
        fb_i = jnp.floor((Hf - fb_h) / 2.0)
        fb_j = jnp.floor((Wf - fb_w) / 2.0)
        final_h = jnp.where(failed, fb_h, final_h)
        final_w = jnp.where(failed, fb_w, final_w)
        final_i = jnp.where(failed, fb_i, final_i)
        final_j = jnp.where(failed, fb_j, final_j)
        i = np.asarray(final_i.astype(jnp.int32))
        j = np.asarray(final_j.astype(jnp.int32))
        h = np.asarray(final_h.astype(jnp.int32))
        w = np.asarray(final_w.astype(jnp.int32))
    return flip_mask, i, j, h, w


def _axis_weights(start, length, n_src_max):
    lf = np.float32(length)
    s = (np.arange(SIZE, dtype=np.float32) + np.float32(0.5)) * lf / np.float32(
        SIZE
    ) - np.float32(0.5)
    s = np.clip(s, np.float32(0.0), lf - np.float32(1.0))
    s0 = np.floor(s)
    frac = (s - s0).astype(np.float32)
    i0 = np.clip(s0.astype(np.int64) + start, 0, n_src_max - 1)
    hi = int(np.clip(start + length - 1, 0, n_src_max - 1))
    i1 = np.clip(i0 + 1, 0, hi)
    return i0, i1, frac


def _build_weights(x):
    """A_yT [B, 512, NP], A_xT [B, 513, NP] f32 (flip folded into A_xT)."""
    Bn = x.shape[0]
    flip_mask, i, j, h, w = _compute_params(x)
    ayt = np.zeros((Bn, HM, NP), dtype=np.float32)
    axt = np.zeros((Bn, WM, NP), dtype=np.float32)
    cols = np.arange(SIZE)
    for b in range(Bn):
        y0, y1, wy = _axis_weights(int(i[b]), int(h[b]), HM)
        np.add.at(ayt[b], (y0, cols), (1.0 - wy))
        np.add.at(ayt[b], (y1, cols), wy)
        x0, x1, wx = _axis_weights(int(j[b]), int(w[b]), HM)
        if flip_mask[b]:
            x0 = HM - x0
            x1 = HM - x1
        np.add.at(axt[b], (x0, cols), (1.0 - wx))
        np.add.at(axt[b], (x1, cols), wx)
    return ayt, axt


# ---------------------------------------------------------------- device prog
def _build_program(use_meta):
    import concourse.mybir as mybir
    import concourse.tile as tile
    from concourse import bacc

    DT = getattr(mybir.dt, DT_NAME)
    F32 = mybir.dt.float32

    nc = bacc.Bacc(None)
    x_d = nc.dram_tensor("x_s", [BPC, C, HM, WM], DT, kind="ExternalInput")
    ayt_d = nc.dram_tensor("ayt", [BPC, HM, NP], DT, kind="ExternalInput")
    axt_d = nc.dram_tensor("axt", [BPC, WM, NP], DT, kind="ExternalInput")
    out_d = nc.dram_tensor("out", [BPC, C, SIZE, SIZE], F32, kind="ExternalOutput")

    with tile.TileContext(nc) as tc:
        with (
            tc.tile_pool(name="img", bufs=4) as img_pool,
            tc.tile_pool(name="wy", bufs=3) as wy_pool,
            tc.tile_pool(name="wx", bufs=3) as wx_pool,
            tc.tile_pool(name="tq", bufs=16) as tq_pool,
            tc.tile_pool(name="ob", bufs=6) as out_pool,
            tc.tile_pool(name="ps1", bufs=4, space="PSUM") as ps1,
            tc.tile_pool(name="ps2", bufs=3, space="PSUM") as ps2,
        ):

            def emit_stage2(job):
                b, c, tqs, tmeta, axt_t, axt_m = job
                for m2 in range(2):
                    m2sz = min(128, SIZE - m2 * 128)
                    sl = slice(m2 * 128, m2 * 128 + m2sz)
                    po = ps2.tile([128, NP], F32, tag="ps2")
                    for k2 in range(KY):
                        nc.tensor.matmul(
                            po[:m2sz, :],
                            tqs[k2][:, sl],
                            axt_t[:, k2, :],
                            start=(k2 == 0),
                            stop=(k2 == KY - 1 and not use_meta),
                        )
                    if use_meta:
                        nc.tensor.matmul(
                            po[:m2sz, :],
                            tmeta[:1, sl],
                            axt_m[:1, :],
                            start=False,
                            stop=True,
                        )
                    ob = out_pool.tile([128, SIZE], F32, tag="ob")
                    nc.vector.tensor_copy(ob[:m2sz, :], po[:m2sz, :SIZE])
                    nc.sync.dma_start(out_d[b, c, sl, :], ob[:m2sz, :])

            prev = None
            for b in range(BPC):
                ayt_t = wy_pool.tile([128, KY, NP], DT, tag="wy")
                nc.sync.dma_start(
                    ayt_t[:], ayt_d[b].rearrange("(a p) n -> p a n", p=128)
                )
                axt_t = wx_pool.tile([128, KY, NP], DT, tag="wx")
                nc.sync.dma_start(
                    axt_t[:], axt_d[b, :HM].rearrange("(a p) n -> p a n", p=128)
                )
                axt_m = None
                if use_meta:
                    axt_m = wx_pool.tile([1, NP], DT, tag="wxm")
                    nc.sync.dma_start(axt_m[:], axt_d[b, HM : HM + 1, :])
                for c in range(C):
                    img_t = img_pool.tile([128, KY, WM], DT, tag="img")
                    xv = x_d[b, c].rearrange("(a p) w -> p a w", p=128)
                    nc.sync.dma_start(img_t[:, 0:2, :], xv[:, 0:2, :])
                    nc.sync.dma_start(img_t[:, 2:4, :], xv[:, 2:4, :])
                    tqs = []
                    for q in range(KY):
                        p1 = ps1.tile([128, NP], F32, tag="ps1")
                        for k in range(KY):
                            nc.tensor.matmul(
                                p1[:, :],
                                img_t[:, k, q * 128 : (q + 1) * 128],
                                ayt_t[:, k, :],
                                start=(k == 0),
                                stop=(k == KY - 1),
                            )
                        tq = tq_pool.tile([128, SIZE], DT, tag="tq")
                        nc.vector.tensor_copy(tq[:], p1[:, :SIZE])
                        tqs.append(tq)
                    tmeta = None
                    if use_meta:
                        pm = ps1.tile([128, NP], F32, tag="ps1")
                        for k in range(KY):
                            nc.tensor.matmul(
                                pm[:1, :],
                                img_t[:, k, HM : HM + 1],
                                ayt_t[:, k, :],
                                start=(k == 0),
                                stop=(k == KY - 1),
                            )
                        tmeta = tq_pool.tile([1, SIZE], DT, tag="tqm")
                        nc.vector.tensor_copy(tmeta[:], pm[:1, :SIZE])
                    if prev is not None:
                        emit_stage2(prev)
                    prev = (b, c, tqs, tmeta, axt_t, axt_m)
            emit_stage2(prev)

    nc.compile()
    return nc


def _get_program(use_meta):
    key = (use_meta, DT_NAME, NP)
    if key not in _prog_cache:
        _prog_cache[key] = _build_program(use_meta)
    return _prog_cache[key]


def _np_dt(name):
    if name == "bfloat16":
        import ml_dtypes

        return ml_dtypes.bfloat16
    return np.float32


def _round_f32r(a):
    """Pre-round fp32 data to the FP32R grid the PE uses, so device inputs
    are already 'rounded to FP32r' and results are deterministic."""
    if F32R_BITS is None or F32R_BITS >= 23:
        return a
    drop = 23 - F32R_BITS
    u = np.ascontiguousarray(a).view(np.uint32)
    if F32R_MODE == "rne":
        half = np.uint32((1 << (drop - 1)) - 1)
        lsb = (u >> drop) & np.uint32(1)
        u = u + half + lsb
    u = (u >> drop) << drop
    return u.view(np.float32)


# ---------------------------------------------------------------- entry point
def kernel(x, trace=False):
    from concourse.bass_utils import run_bass_kernel_spmd

    x = np.ascontiguousarray(np.asarray(x, dtype=np.float32))
    assert x.shape == (B_FULL, C, HM, WM), x.shape

    ayt, axt = _build_weights(x)
    use_meta = bool(np.abs(axt[:, HM, :]).max() > 0)

    nc = _get_program(use_meta)

    ndt = _np_dt(DT_NAME)
    xc = x.astype(ndt, copy=False)
    aytc = ayt.astype(ndt, copy=False)
    axtc = axt.astype(ndt, copy=False)
    if DT_NAME == "float32r":
        xc = _round_f32r(xc)
        aytc = _round_f32r(aytc)
        axtc = _round_f32r(axtc)

    in_maps = []
    for k in range(N_CORES):
        sl = slice(k * BPC, (k + 1) * BPC)
        in_maps.append(
            {
                "x_s": np.ascontiguousarray(xc[sl]),
                "ayt": np.ascontiguousarray(aytc[sl]),
                "axt": np.ascontiguousarray(axtc[sl]),
            }
        )

    res = run_bass_kernel_spmd(nc, in_maps, list(range(N_CORES)), trace=trace)
    out = np.concatenate([res.results[k]["out"] for k in range(N_CORES)], axis=0)
    if trace:
        kernel.last_exec_ns = res.exec_time_ns
        kernel.last_results = res
    return out


# revision 9
# speedup vs baseline: 1.9531x; 1.2783x over previous
"""Trainium2 Bass kernel for nn_DataAugmentation (RandomResizedCrop + hflip batch).

Strategy
--------
Data parallel over batch: core k handles samples [8k, 8k+8).

All data-dependent work (RNG replication, crop params, bilinear weights,
hflip column remap) is tiny and happens on host; it is encoded into two
small per-sample interpolation matrices so the device does only dense
matmuls over its full input shard:

    out[b, c] = (A_yT[b].T @ img[b, c]) @ A_xT[b]
              = stage1 (row interp)      stage2 (col interp)

with img the ORIGINAL image (incl. metadata col, 512x513); the horizontal
flip is absorbed into A_xT's source-column index map.

Device stage 1 computes t.T = img.T @ A_yT chunk-wise (lhsT = img slice —
natural row-major layout, no transposes anywhere), stage 2 computes
out = (t.T).T @ A_xT.
"""

import numpy as np

SIZE = 224
HM = 512
WM = 513  # includes metadata column
B_FULL = 64
C = 3
N_CORES = 8
BPC = B_FULL // N_CORES  # samples per core
KY = HM // 128  # k-tiles over rows / cols

# dtype config: "float32" (exact), "float16", "bfloat16", "float32r"
DT_NAME = "float16"
NP = 224  # padded free dim (>=224; must be 256 for float32r full-rate matmuls)
F32R_BITS = 11  # HW-probed: FP32R keeps 11 mantissa bits, round-nearest-even
F32R_MODE = "rne"

SCALE = (0.1, 1.0)
RATIO = (0.8, 1.25)
N_TRIES = 10

_prog_cache = {}


# ---------------------------------------------------------------- host math
def _compute_params(x):
    """Replicates reference._get_params + flip RNG exactly (jax on CPU)."""
    import jax
    import jax.numpy as jnp

    cpu = jax.devices("cpu")[0]
    with jax.default_device(cpu):
        B = x.shape[0]
        H = x[:, 0, 0, -1].astype(np.int32)
        W = x[:, 1, 0, -1].astype(np.int32)
        key = jax.random.key(42)
        kflip, kparams = jax.random.split(key)
        flip_mask = np.asarray(jax.random.uniform(kflip, (B,)) > 0.5)

        Hf = jnp.asarray(H, jnp.float32)
        Wf = jnp.asarray(W, jnp.float32)
        area = Hf * Wf
        log_lo = np.log(RATIO[0]).astype(np.float32)
        log_hi = np.log(RATIO[1]).astype(np.float32)
        final_h = jnp.full((B,), -1.0, dtype=jnp.float32)
        final_w = jnp.full((B,), -1.0, dtype=jnp.float32)
        final_i = jnp.full((B,), -1.0, dtype=jnp.float32)
        final_j = jnp.full((B,), -1.0, dtype=jnp.float32)
        success = jnp.zeros((B,), dtype=bool)
        for t in range(N_TRIES):
            k1, k2, k3, k4 = jax.random.split(jax.random.fold_in(kparams, t), 4)
            target_area = area * jax.random.uniform(
                k1, (B,), minval=SCALE[0], maxval=SCALE[1]
            )
            aspect = jnp.exp(jax.random.uniform(k2, (B,), minval=log_lo, maxval=log_hi))
            crop_w = jnp.round(jnp.sqrt(target_area * aspect))
            crop_h = jnp.round(jnp.sqrt(target_area / aspect))
            valid = (
                (crop_w > 0) & (crop_w <= Wf) & (crop_h > 0) & (crop_h <= Hf) & (~success)
            )
            max_i = jnp.clip(Hf - crop_h + 1.0, 1.0, None)
            max_j = jnp.clip(Wf - crop_w + 1.0, 1.0, None)
            rand_i = jnp.floor(jax.random.uniform(k3, (B,)) * max_i)
            rand_j = jnp.floor(jax.random.uniform(k4, (B,)) * max_j)
            final_h = jnp.where(valid, crop_h, final_h)
            final_w = jnp.where(valid, crop_w, final_w)
            final_i = jnp.where(valid, rand_i, final_i)
            final_j = jnp.where(valid, rand_j, final_j)
            success = success | valid
        failed = ~success
        in_ratio = Wf / Hf
        fb_w = jnp.where(in_ratio > RATIO[1], jnp.round(Hf * RATIO[1]), Wf)
        fb_h = jnp.where(in_ratio < RATIO[0], jnp.round(Wf / RATIO[0]), Hf)
        fb_i = jnp.floor((Hf - fb_h) / 2.0)
        fb_j = jnp.floor((Wf - fb_w) / 2.0)
        final_h = jnp.where(failed, fb_h, final_h)
        final_w = jnp.where(failed, fb_w, final_w)
        final_i = jnp.where(failed, fb_i, final_i)
        final_j = jnp.where(failed, fb_j, final_j)
        i = np.asarray(final_i.astype(jnp.int32))
        j = np.asarray(final_j.astype(jnp.int32))
        h = np.asarray(final_h.astype(jnp.int32))
        w = np.asarray(final_w.astype(jnp.int32))
    return flip_mask, i, j, h, w


def _axis_weights(start, length, n_src_max):
    lf = np.float32(length)
    s = (np.arange(SIZE, dtype=np.float32) + np.float32(0.5)) * lf / np.float32(
        SIZE
    ) - np.float32(0.5)
    s = np.clip(s, np.float32(0.0), lf - np.float32(1.0))
    s0 = np.floor(s)
    frac = (s - s0).astype(np.float32)
    i0 = np.clip(s0.astype(np.int64) + start, 0, n_src_max - 1)
    hi = int(np.clip(start + length - 1, 0, n_src_max - 1))
    i1 = np.clip(i0 + 1, 0, hi)
    return i0, i1, frac


def _build_weights(x):
    """A_yT [B, 512, NP], A_xT [B, 513, NP] f32 (flip folded into A_xT)."""
    Bn = x.shape[0]
    flip_mask, i, j, h, w = _compute_params(x)
    ayt = np.zeros((Bn, HM, NP), dtype=np.float32)
    axt = np.zeros((Bn, WM, NP), dtype=np.float32)
    cols = np.arange(SIZE)
    for b in range(Bn):
        y0, y1, wy = _axis_weights(int(i[b]), int(h[b]), HM)
        np.add.at(ayt[b], (y0, cols), (1.0 - wy))
        np.add.at(ayt[b], (y1, cols), wy)
        x0, x1, wx = _axis_weights(int(j[b]), int(w[b]), HM)
        if flip_mask[b]:
            x0 = HM - x0
            x1 = HM - x1
        np.add.at(axt[b], (x0, cols), (1.0 - wx))
        np.add.at(axt[b], (x1, cols), wx)
    return ayt, axt


# ---------------------------------------------------------------- device prog
def _build_program(use_meta):
    import concourse.mybir as mybir
    import concourse.tile as tile
    from concourse import bacc

    DT = getattr(mybir.dt, DT_NAME)
    F32 = mybir.dt.float32

    nc = bacc.Bacc(None)
    x_d = nc.dram_tensor("x_s", [BPC, C, HM, WM], DT, kind="ExternalInput")
    ayt_d = nc.dram_tensor("ayt", [BPC, HM, NP], DT, kind="ExternalInput")
    axt_d = nc.dram_tensor("axt", [BPC, WM, NP], DT, kind="ExternalInput")
    out_d = nc.dram_tensor("out", [BPC, C, SIZE, SIZE], F32, kind="ExternalOutput")

    with tile.TileContext(nc) as tc:
        with (
            tc.tile_pool(name="img", bufs=4) as img_pool,
            tc.tile_pool(name="wy", bufs=3) as wy_pool,
            tc.tile_pool(name="wx", bufs=3) as wx_pool,
            tc.tile_pool(name="tq", bufs=16) as tq_pool,
            tc.tile_pool(name="ob", bufs=6) as out_pool,
            tc.tile_pool(name="ps1", bufs=4, space="PSUM") as ps1,
            tc.tile_pool(name="ps2", bufs=3, space="PSUM") as ps2,
        ):

            def emit_stage2(job):
                b, c, tqs, tmeta, axt_t, axt_m = job
                for m2 in range(2):
                    m2sz = min(128, SIZE - m2 * 128)
                    sl = slice(m2 * 128, m2 * 128 + m2sz)
                    po = ps2.tile([128, NP], F32, tag="ps2")
                    for k2 in range(KY):
                        nc.tensor.matmul(
                            po[:m2sz, :],
                            tqs[k2][:, sl],
                            axt_t[:, k2, :],
                            start=(k2 == 0),
                            stop=(k2 == KY - 1 and not use_meta),
                        )
                    if use_meta:
                        nc.tensor.matmul(
                            po[:m2sz, :],
                            tmeta[:1, sl],
                            axt_m[:1, :],
                            start=False,
                            stop=True,
                        )
                    ob = out_pool.tile([128, SIZE], F32, tag="ob")
                    nc.vector.tensor_copy(ob[:m2sz, :], po[:m2sz, :SIZE])
                    nc.sync.dma_start(out_d[b, c, sl, :], ob[:m2sz, :])

            prev = None
            for b in range(BPC):
                ayt_t = wy_pool.tile([128, KY, NP], DT, tag="wy")
                nc.sync.dma_start(
                    ayt_t[:], ayt_d[b].rearrange("(a p) n -> p a n", p=128)
                )
                axt_t = wx_pool.tile([128, KY, NP], DT, tag="wx")
                nc.sync.dma_start(
                    axt_t[:], axt_d[b, :HM].rearrange("(a p) n -> p a n", p=128)
                )
                axt_m = None
                if use_meta:
                    axt_m = wx_pool.tile([1, NP], DT, tag="wxm")
                    nc.sync.dma_start(axt_m[:], axt_d[b, HM : HM + 1, :])
                for c in range(C):
                    img_t = img_pool.tile([128, KY, WM], DT, tag="img")
                    xv = x_d[b, c].rearrange("(a p) w -> p a w", p=128)
                    nc.sync.dma_start(img_t[:, 0:2, :], xv[:, 0:2, :])
                    nc.sync.dma_start(img_t[:, 2:4, :], xv[:, 2:4, :])
                    tqs = []
                    for q in range(KY):
                        p1 = ps1.tile([128, NP], F32, tag="ps1")
                        for k in range(KY):
                            nc.tensor.matmul(
                                p1[:, :],
                                img_t[:, k, q * 128 : (q + 1) * 128],
                                ayt_t[:, k, :],
                                start=(k == 0),
                                stop=(k == KY - 1),
                            )
                        tq = tq_pool.tile([128, SIZE], DT, tag="tq")
                        nc.vector.tensor_copy(tq[:], p1[:, :SIZE])
                        tqs.append(tq)
                    tmeta = None
                    if use_meta:
                        pm = ps1.tile([128, NP], F32, tag="ps1")
                        for k in range(KY):
                            nc.tensor.matmul(
                                pm[:1, :],
                                img_t[:, k, HM : HM + 1],
                                ayt_t[:, k, :],
                                start=(k == 0),
                                stop=(k == KY - 1),
                            )
                        tmeta = tq_pool.tile([1, SIZE], DT, tag="tqm")
                        nc.vector.tensor_copy(tmeta[:], pm[:1, :SIZE])
                    if prev is not None:
                        emit_stage2(prev)
                    prev = (b, c, tqs, tmeta, axt_t, axt_m)
            emit_stage2(prev)

    nc.compile()
    return nc


def _get_program(use_meta):
    key = (use_meta, DT_NAME, NP)
    if key not in _prog_cache:
        _prog_cache[key] = _build_program(use_meta)
    return _prog_cache[key]


def _np_dt(name):
    if name == "bfloat16":
        import ml_dtypes

        return ml_dtypes.bfloat16
    if name == "float16":
        return np.float16
    return np.float32


def _round_f32r(a):
    """Pre-round fp32 data to the FP32R grid the PE uses, so device inputs
    are already 'rounded to FP32r' and results are deterministic."""
    if F32R_BITS is None or F32R_BITS >= 23:
        return a
    drop = 23 - F32R_BITS
    u = np.ascontiguousarray(a).view(np.uint32)
    if F32R_MODE == "rne":
        half = np.uint32((1 << (drop - 1)) - 1)
        lsb = (u >> drop) & np.uint32(1)
        u = u + half + lsb
    u = (u >> drop) << drop
    return u.view(np.float32)


# ---------------------------------------------------------------- entry point
def kernel(x, trace=False):
    from concourse.bass_utils import run_bass_kernel_spmd

    x = np.ascontiguousarray(np.asarray(x, dtype=np.float32))
    assert x.shape == (B_FULL, C, HM, WM), x.shape

    ayt, axt = _build_weights(x)
    use_meta = bool(np.abs(axt[:, HM, :]).max() > 0)

    nc = _get_program(use_meta)

    ndt = _np_dt(DT_NAME)
    xc = x.astype(ndt, copy=False)
    aytc = ayt.astype(ndt, copy=False)
    axtc = axt.astype(ndt, copy=False)
    if DT_NAME == "float32r":
        xc = _round_f32r(xc)
        aytc = _round_f32r(aytc)
        axtc = _round_f32r(axtc)

    in_maps = []
    for k in range(N_CORES):
        sl = slice(k * BPC, (k + 1) * BPC)
        in_maps.append(
            {
                "x_s": np.ascontiguousarray(xc[sl]),
                "ayt": np.ascontiguousarray(aytc[sl]),
                "axt": np.ascontiguousarray(axtc[sl]),
            }
        )

    res = run_bass_kernel_spmd(nc, in_maps, list(range(N_CORES)), trace=trace)
    out = np.concatenate([res.results[k]["out"] for k in range(N_CORES)], axis=0)
    if trace:
        kernel.last_exec_ns = res.exec_time_ns
        kernel.last_results = res
    return out


# revision 11
# speedup vs baseline: 2.0045x; 1.0263x over previous
"""Trainium2 Bass kernel for nn_DataAugmentation (RandomResizedCrop + hflip batch).

Strategy
--------
Data parallel over batch: core k handles samples [8k, 8k+8).

All data-dependent work (RNG replication, crop params, bilinear weights,
hflip column remap) is tiny and happens on host; it is encoded into two
small per-sample interpolation matrices so the device does only dense
matmuls over its full input shard:

    out[b, c] = (A_yT[b].T @ img[b, c]) @ A_xT[b]
              = stage1 (row interp)      stage2 (col interp)

with img the ORIGINAL image (incl. metadata col, 512x513); the horizontal
flip is absorbed into A_xT's source-column index map.

Device stage 1 computes t.T = img.T @ A_yT chunk-wise (lhsT = img slice —
natural row-major layout, no transposes anywhere), stage 2 computes
out = (t.T).T @ A_xT.
"""

import numpy as np

SIZE = 224
HM = 512
WM = 513  # includes metadata column
B_FULL = 64
C = 3
N_CORES = 8
BPC = B_FULL // N_CORES  # samples per core
KY = HM // 128  # k-tiles over rows / cols

# dtype config: "float32" (exact), "float16", "bfloat16", "float32r"
DT_NAME = "float16"
NP = 224  # padded free dim (>=224; must be 256 for float32r full-rate matmuls)
F32R_BITS = 11  # HW-probed: FP32R keeps 11 mantissa bits, round-nearest-even
F32R_MODE = "rne"

SCALE = (0.1, 1.0)
RATIO = (0.8, 1.25)
N_TRIES = 10

_prog_cache = {}


# ---------------------------------------------------------------- host math
def _compute_params(x):
    """Replicates reference._get_params + flip RNG exactly (jax on CPU)."""
    import jax
    import jax.numpy as jnp

    cpu = jax.devices("cpu")[0]
    with jax.default_device(cpu):
        B = x.shape[0]
        H = x[:, 0, 0, -1].astype(np.int32)
        W = x[:, 1, 0, -1].astype(np.int32)
        key = jax.random.key(42)
        kflip, kparams = jax.random.split(key)
        flip_mask = np.asarray(jax.random.uniform(kflip, (B,)) > 0.5)

        Hf = jnp.asarray(H, jnp.float32)
        Wf = jnp.asarray(W, jnp.float32)
        area = Hf * Wf
        log_lo = np.log(RATIO[0]).astype(np.float32)
        log_hi = np.log(RATIO[1]).astype(np.float32)
        final_h = jnp.full((B,), -1.0, dtype=jnp.float32)
        final_w = jnp.full((B,), -1.0, dtype=jnp.float32)
        final_i = jnp.full((B,), -1.0, dtype=jnp.float32)
        final_j = jnp.full((B,), -1.0, dtype=jnp.float32)
        success = jnp.zeros((B,), dtype=bool)
        for t in range(N_TRIES):
            k1, k2, k3, k4 = jax.random.split(jax.random.fold_in(kparams, t), 4)
            target_area = area * jax.random.uniform(
                k1, (B,), minval=SCALE[0], maxval=SCALE[1]
            )
            aspect = jnp.exp(jax.random.uniform(k2, (B,), minval=log_lo, maxval=log_hi))
            crop_w = jnp.round(jnp.sqrt(target_area * aspect))
            crop_h = jnp.round(jnp.sqrt(target_area / aspect))
            valid = (
                (crop_w > 0) & (crop_w <= Wf) & (crop_h > 0) & (crop_h <= Hf) & (~success)
            )
            max_i = jnp.clip(Hf - crop_h + 1.0, 1.0, None)
            max_j = jnp.clip(Wf - crop_w + 1.0, 1.0, None)
            rand_i = jnp.floor(jax.random.uniform(k3, (B,)) * max_i)
            rand_j = jnp.floor(jax.random.uniform(k4, (B,)) * max_j)
            final_h = jnp.where(valid, crop_h, final_h)
            final_w = jnp.where(valid, crop_w, final_w)
            final_i = jnp.where(valid, rand_i, final_i)
            final_j = jnp.where(valid, rand_j, final_j)
            success = success | valid
        failed = ~success
        in_ratio = Wf / Hf
        fb_w = jnp.where(in_ratio > RATIO[1], jnp.round(Hf * RATIO[1]), Wf)
        fb_h = jnp.where(in_ratio < RATIO[0], jnp.round(Wf / RATIO[0]), Hf)
        fb_i = jnp.floor((Hf - fb_h) / 2.0)
        fb_j = jnp.floor((Wf - fb_w) / 2.0)
        final_h = jnp.where(failed, fb_h, final_h)
        final_w = jnp.where(failed, fb_w, final_w)
        final_i = jnp.where(failed, fb_i, final_i)
        final_j = jnp.where(failed, fb_j, final_j)
        i = np.asarray(final_i.astype(jnp.int32))
        j = np.asarray(final_j.astype(jnp.int32))
        h = np.asarray(final_h.astype(jnp.int32))
        w = np.asarray(final_w.astype(jnp.int32))
    return flip_mask, i, j, h, w


def _axis_weights(start, length, n_src_max):
    lf = np.float32(length)
    s = (np.arange(SIZE, dtype=np.float32) + np.float32(0.5)) * lf / np.float32(
        SIZE
    ) - np.float32(0.5)
    s = np.clip(s, np.float32(0.0), lf - np.float32(1.0))
    s0 = np.floor(s)
    frac = (s - s0).astype(np.float32)
    i0 = np.clip(s0.astype(np.int64) + start, 0, n_src_max - 1)
    hi = int(np.clip(start + length - 1, 0, n_src_max - 1))
    i1 = np.clip(i0 + 1, 0, hi)
    return i0, i1, frac


def _build_weights(x):
    """A_yT [B, 512, NP], A_xT [B, 513, NP] f32 (flip folded into A_xT)."""
    Bn = x.shape[0]
    flip_mask, i, j, h, w = _compute_params(x)
    ayt = np.zeros((Bn, HM, NP), dtype=np.float32)
    axt = np.zeros((Bn, WM, NP), dtype=np.float32)
    cols = np.arange(SIZE)
    for b in range(Bn):
        y0, y1, wy = _axis_weights(int(i[b]), int(h[b]), HM)
        np.add.at(ayt[b], (y0, cols), (1.0 - wy))
        np.add.at(ayt[b], (y1, cols), wy)
        x0, x1, wx = _axis_weights(int(j[b]), int(w[b]), HM)
        if flip_mask[b]:
            x0 = HM - x0
            x1 = HM - x1
        np.add.at(axt[b], (x0, cols), (1.0 - wx))
        np.add.at(axt[b], (x1, cols), wx)
    return ayt, axt


# ---------------------------------------------------------------- device prog
def _build_program(use_meta):
    import concourse.mybir as mybir
    import concourse.tile as tile
    from concourse import bacc

    DT = getattr(mybir.dt, DT_NAME)
    F32 = mybir.dt.float32

    nc = bacc.Bacc(None)
    x_d = nc.dram_tensor("x_s", [BPC, C, HM, WM], DT, kind="ExternalInput")
    ayt_d = nc.dram_tensor("ayt", [BPC, HM, NP], DT, kind="ExternalInput")
    axt_d = nc.dram_tensor("axt", [BPC, WM, NP], DT, kind="ExternalInput")
    out_d = nc.dram_tensor("out", [BPC, C, SIZE, SIZE], F32, kind="ExternalOutput")

    with tile.TileContext(nc) as tc:
        with (
            tc.tile_pool(name="img", bufs=6) as img_pool,
            tc.tile_pool(name="wy", bufs=4) as wy_pool,
            tc.tile_pool(name="wx", bufs=4) as wx_pool,
            tc.tile_pool(name="tq", bufs=20) as tq_pool,
            tc.tile_pool(name="ob", bufs=8) as out_pool,
            tc.tile_pool(name="ps1", bufs=4, space="PSUM") as ps1,
            tc.tile_pool(name="ps2", bufs=4, space="PSUM") as ps2,
        ):

            def emit_stage2(job):
                b, c, tqs, tmeta, axt_t, axt_m = job
                for m2 in range(2):
                    m2sz = min(128, SIZE - m2 * 128)
                    sl = slice(m2 * 128, m2 * 128 + m2sz)
                    po = ps2.tile([128, NP], F32, tag="ps2")
                    for k2 in range(KY):
                        nc.tensor.matmul(
                            po[:m2sz, :],
                            tqs[k2][:, sl],
                            axt_t[:, k2, :],
                            start=(k2 == 0),
                            stop=(k2 == KY - 1 and not use_meta),
                        )
                    if use_meta:
                        nc.tensor.matmul(
                            po[:m2sz, :],
                            tmeta[:1, sl],
                            axt_m[:1, :],
                            start=False,
                            stop=True,
                        )
                    ob = out_pool.tile([128, SIZE], F32, tag="ob")
                    nc.scalar.copy(ob[:m2sz, :], po[:m2sz, :SIZE])
                    nc.sync.dma_start(out_d[b, c, sl, :], ob[:m2sz, :])

            prev = None
            for b in range(BPC):
                ayt_t = wy_pool.tile([128, KY, NP], DT, tag="wy")
                nc.sync.dma_start(
                    ayt_t[:], ayt_d[b].rearrange("(a p) n -> p a n", p=128)
                )
                axt_t = wx_pool.tile([128, KY, NP], DT, tag="wx")
                nc.sync.dma_start(
                    axt_t[:], axt_d[b, :HM].rearrange("(a p) n -> p a n", p=128)
                )
                axt_m = None
                if use_meta:
                    axt_m = wx_pool.tile([1, NP], DT, tag="wxm")
                    nc.sync.dma_start(axt_m[:], axt_d[b, HM : HM + 1, :])
                for c in range(C):
                    img_t = img_pool.tile([128, KY, WM], DT, tag="img")
                    xv = x_d[b, c].rearrange("(a p) w -> p a w", p=128)
                    nc.sync.dma_start(img_t[:, 0:2, :], xv[:, 0:2, :])
                    nc.sync.dma_start(img_t[:, 2:4, :], xv[:, 2:4, :])
                    tqs = []
                    for q in range(KY):
                        p1 = ps1.tile([128, NP], F32, tag="ps1")
                        for k in range(KY):
                            nc.tensor.matmul(
                                p1[:, :],
                                img_t[:, k, q * 128 : (q + 1) * 128],
                                ayt_t[:, k, :],
                                start=(k == 0),
                                stop=(k == KY - 1),
                            )
                        tq = tq_pool.tile([128, SIZE], DT, tag="tq")
                        nc.vector.tensor_copy(tq[:], p1[:, :SIZE])
                        tqs.append(tq)
                    tmeta = None
                    if use_meta:
                        pm = ps1.tile([128, NP], F32, tag="ps1")
                        for k in range(KY):
                            nc.tensor.matmul(
                                pm[:1, :],
                                img_t[:, k, HM : HM + 1],
                                ayt_t[:, k, :],
                                start=(k == 0),
                                stop=(k == KY - 1),
                            )
                        tmeta = tq_pool.tile([1, SIZE], DT, tag="tqm")
                        nc.vector.tensor_copy(tmeta[:], pm[:1, :SIZE])
                    if prev is not None:
                        emit_stage2(prev)
                    prev = (b, c, tqs, tmeta, axt_t, axt_m)
            emit_stage2(prev)

    nc.compile()
    return nc


def _get_program(use_meta):
    key = (use_meta, DT_NAME, NP)
    if key not in _prog_cache:
        _prog_cache[key] = _build_program(use_meta)
    return _prog_cache[key]


def _np_dt(name):
    if name == "bfloat16":
        import ml_dtypes

        return ml_dtypes.bfloat16
    if name == "float16":
        return np.float16
    return np.float32


def _round_f32r(a):
    """Pre-round fp32 data to the FP32R grid the PE uses, so device inputs
    are already 'rounded to FP32r' and results are deterministic."""
    if F32R_BITS is None or F32R_BITS >= 23:
        return a
    drop = 23 - F32R_BITS
    u = np.ascontiguousarray(a).view(np.uint32)
    if F32R_MODE == "rne":
        half = np.uint32((1 << (drop - 1)) - 1)
        lsb = (u >> drop) & np.uint32(1)
        u = u + half + lsb
    u = (u >> drop) << drop
    return u.view(np.float32)


# ---------------------------------------------------------------- entry point
def kernel(x, trace=False):
    from concourse.bass_utils import run_bass_kernel_spmd

    x = np.ascontiguousarray(np.asarray(x, dtype=np.float32))
    assert x.shape == (B_FULL, C, HM, WM), x.shape

    ayt, axt = _build_weights(x)
    use_meta = bool(np.abs(axt[:, HM, :]).max() > 0)

    nc = _get_program(use_meta)

    ndt = _np_dt(DT_NAME)
    xc = x.astype(ndt, copy=False)
    aytc = ayt.astype(ndt, copy=False)
    axtc = axt.astype(ndt, copy=False)
    if DT_NAME == "float32r":
        xc = _round_f32r(xc)
        aytc = _round_f32r(aytc)
        axtc = _round_f32r(axtc)

    in_maps = []
    for k in range(N_CORES):
        sl = slice(k * BPC, (k + 1) * BPC)
        in_maps.append(
            {
                "x_s": np.ascontiguousarray(xc[sl]),
                "ayt": np.ascontiguousarray(aytc[sl]),
                "axt": np.ascontiguousarray(axtc[sl]),
            }
        )

    res = run_bass_kernel_spmd(nc, in_maps, list(range(N_CORES)), trace=trace)
    out = np.concatenate([res.results[k]["out"] for k in range(N_CORES)], axis=0)
    if trace:
        kernel.last_exec_ns = res.exec_time_ns
        kernel.last_results = res
    return out


# revision 14
# speedup vs baseline: 2.1685x; 1.0819x over previous
"""Trainium2 Bass kernel for nn_DataAugmentation (RandomResizedCrop + hflip batch).

Strategy
--------
Data parallel over batch: core k handles samples [8k, 8k+8).

All data-dependent work (RNG replication, crop params, bilinear weights,
hflip column remap) is tiny and happens on host; it is encoded into two
small per-sample interpolation matrices so the device does only dense
matmuls over its full input shard:

    out[b, c] = (A_yT[b].T @ img[b, c]) @ A_xT[b]
              = stage1 (row interp)      stage2 (col interp)

with img the ORIGINAL image (incl. metadata col, 512x513); the horizontal
flip is absorbed into A_xT's source-column index map.

Device stage 1 computes t.T = img.T @ A_yT chunk-wise (lhsT = img slice —
natural row-major layout, no transposes anywhere), stage 2 computes
out = (t.T).T @ A_xT.
"""

import numpy as np

SIZE = 224
HM = 512
WM = 513  # includes metadata column
B_FULL = 64
C = 3
N_CORES = 8
BPC = B_FULL // N_CORES  # samples per core
KY = HM // 128  # k-tiles over rows / cols

# dtype config: "float32" (exact), "float16", "bfloat16", "float32r"
DT_NAME = "float16"
NP = 224  # padded free dim (>=224; must be 256 for float32r full-rate matmuls)
F32R_BITS = 11  # HW-probed: FP32R keeps 11 mantissa bits, round-nearest-even
F32R_MODE = "rne"

SCALE = (0.1, 1.0)
RATIO = (0.8, 1.25)
N_TRIES = 10

_prog_cache = {}


# ---------------------------------------------------------------- host math
def _compute_params(x):
    """Replicates reference._get_params + flip RNG exactly (jax on CPU)."""
    import jax
    import jax.numpy as jnp

    cpu = jax.devices("cpu")[0]
    with jax.default_device(cpu):
        B = x.shape[0]
        H = x[:, 0, 0, -1].astype(np.int32)
        W = x[:, 1, 0, -1].astype(np.int32)
        key = jax.random.key(42)
        kflip, kparams = jax.random.split(key)
        flip_mask = np.asarray(jax.random.uniform(kflip, (B,)) > 0.5)

        Hf = jnp.asarray(H, jnp.float32)
        Wf = jnp.asarray(W, jnp.float32)
        area = Hf * Wf
        log_lo = np.log(RATIO[0]).astype(np.float32)
        log_hi = np.log(RATIO[1]).astype(np.float32)
        final_h = jnp.full((B,), -1.0, dtype=jnp.float32)
        final_w = jnp.full((B,), -1.0, dtype=jnp.float32)
        final_i = jnp.full((B,), -1.0, dtype=jnp.float32)
        final_j = jnp.full((B,), -1.0, dtype=jnp.float32)
        success = jnp.zeros((B,), dtype=bool)
        for t in range(N_TRIES):
            k1, k2, k3, k4 = jax.random.split(jax.random.fold_in(kparams, t), 4)
            target_area = area * jax.random.uniform(
                k1, (B,), minval=SCALE[0], maxval=SCALE[1]
            )
            aspect = jnp.exp(jax.random.uniform(k2, (B,), minval=log_lo, maxval=log_hi))
            crop_w = jnp.round(jnp.sqrt(target_area * aspect))
            crop_h = jnp.round(jnp.sqrt(target_area / aspect))
            valid = (
                (crop_w > 0) & (crop_w <= Wf) & (crop_h > 0) & (crop_h <= Hf) & (~success)
            )
            max_i = jnp.clip(Hf - crop_h + 1.0, 1.0, None)
            max_j = jnp.clip(Wf - crop_w + 1.0, 1.0, None)
            rand_i = jnp.floor(jax.random.uniform(k3, (B,)) * max_i)
            rand_j = jnp.floor(jax.random.uniform(k4, (B,)) * max_j)
            final_h = jnp.where(valid, crop_h, final_h)
            final_w = jnp.where(valid, crop_w, final_w)
            final_i = jnp.where(valid, rand_i, final_i)
            final_j = jnp.where(valid, rand_j, final_j)
            success = success | valid
        failed = ~success
        in_ratio = Wf / Hf
        fb_w = jnp.where(in_ratio > RATIO[1], jnp.round(Hf * RATIO[1]), Wf)
        fb_h = jnp.where(in_ratio < RATIO[0], jnp.round(Wf / RATIO[0]), Hf)
        fb_i = jnp.floor((Hf - fb_h) / 2.0)
        fb_j = jnp.floor((Wf - fb_w) / 2.0)
        final_h = jnp.where(failed, fb_h, final_h)
        final_w = jnp.where(failed, fb_w, final_w)
        final_i = jnp.where(failed, fb_i, final_i)
        final_j = jnp.where(failed, fb_j, final_j)
        i = np.asarray(final_i.astype(jnp.int32))
        j = np.asarray(final_j.astype(jnp.int32))
        h = np.asarray(final_h.astype(jnp.int32))
        w = np.asarray(final_w.astype(jnp.int32))
    return flip_mask, i, j, h, w


def _axis_weights(start, length, n_src_max):
    lf = np.float32(length)
    s = (np.arange(SIZE, dtype=np.float32) + np.float32(0.5)) * lf / np.float32(
        SIZE
    ) - np.float32(0.5)
    s = np.clip(s, np.float32(0.0), lf - np.float32(1.0))
    s0 = np.floor(s)
    frac = (s - s0).astype(np.float32)
    i0 = np.clip(s0.astype(np.int64) + start, 0, n_src_max - 1)
    hi = int(np.clip(start + length - 1, 0, n_src_max - 1))
    i1 = np.clip(i0 + 1, 0, hi)
    return i0, i1, frac


def _build_weights(x):
    """A_yT [B, 512, NP], A_xT [B, 513, NP] f32 (flip folded into A_xT)."""
    Bn = x.shape[0]
    flip_mask, i, j, h, w = _compute_params(x)
    ayt = np.zeros((Bn, HM, NP), dtype=np.float32)
    axt = np.zeros((Bn, WM, NP), dtype=np.float32)
    cols = np.arange(SIZE)
    for b in range(Bn):
        y0, y1, wy = _axis_weights(int(i[b]), int(h[b]), HM)
        np.add.at(ayt[b], (y0, cols), (1.0 - wy))
        np.add.at(ayt[b], (y1, cols), wy)
        x0, x1, wx = _axis_weights(int(j[b]), int(w[b]), HM)
        if flip_mask[b]:
            x0 = HM - x0
            x1 = HM - x1
        np.add.at(axt[b], (x0, cols), (1.0 - wx))
        np.add.at(axt[b], (x1, cols), wx)
    return ayt, axt


# ---------------------------------------------------------------- device prog
def _build_program(use_meta):
    import concourse.mybir as mybir
    import concourse.tile as tile
    from concourse import bacc

    DT = getattr(mybir.dt, DT_NAME)
    F32 = mybir.dt.float32

    nc = bacc.Bacc(None)
    x_d = nc.dram_tensor("x_s", [BPC, C, HM, WM], DT, kind="ExternalInput")
    ayt_d = nc.dram_tensor("ayt", [BPC, HM, NP], DT, kind="ExternalInput")
    axt_d = nc.dram_tensor("axt", [BPC, WM, NP], DT, kind="ExternalInput")
    out_d = nc.dram_tensor("out", [BPC, C, SIZE, SIZE], F32, kind="ExternalOutput")

    with tile.TileContext(nc) as tc:
        with (
            tc.tile_pool(name="img", bufs=8) as img_pool,
            tc.tile_pool(name="wy", bufs=4) as wy_pool,
            tc.tile_pool(name="wx", bufs=4) as wx_pool,
            tc.tile_pool(name="tq", bufs=20) as tq_pool,
            tc.tile_pool(name="ob", bufs=8) as out_pool,
            tc.tile_pool(name="ps1", bufs=4, space="PSUM") as ps1,
            tc.tile_pool(name="ps2", bufs=4, space="PSUM") as ps2,
        ):

            def emit_stage2(job):
                b, c, tqs, tmeta, axt_t, axt_m = job
                for m2 in range(2):
                    m2sz = min(128, SIZE - m2 * 128)
                    sl = slice(m2 * 128, m2 * 128 + m2sz)
                    po = ps2.tile([128, NP], F32, tag="ps2")
                    for k2 in range(KY):
                        nc.tensor.matmul(
                            po[:m2sz, :],
                            tqs[k2][:, sl],
                            axt_t[:, k2, :],
                            start=(k2 == 0),
                            stop=(k2 == KY - 1 and not use_meta),
                        )
                    if use_meta:
                        nc.tensor.matmul(
                            po[:m2sz, :],
                            tmeta[:1, sl],
                            axt_m[:1, :],
                            start=False,
                            stop=True,
                        )
                    ob = out_pool.tile([128, SIZE], F32, tag="ob")
                    nc.scalar.copy(ob[:m2sz, :], po[:m2sz, :SIZE])
                    nc.sync.dma_start(out_d[b, c, sl, :], ob[:m2sz, :])

            pending = []
            PIPE_DEPTH = 2
            for b in range(BPC):
                ayt_t = wy_pool.tile([128, KY, NP], DT, tag="wy")
                nc.sync.dma_start(
                    ayt_t[:], ayt_d[b].rearrange("(a p) n -> p a n", p=128)
                )
                axt_t = wx_pool.tile([128, KY, NP], DT, tag="wx")
                nc.sync.dma_start(
                    axt_t[:], axt_d[b, :HM].rearrange("(a p) n -> p a n", p=128)
                )
                axt_m = None
                if use_meta:
                    axt_m = wx_pool.tile([1, NP], DT, tag="wxm")
                    nc.sync.dma_start(axt_m[:], axt_d[b, HM : HM + 1, :])
                for c in range(C):
                    img_t = img_pool.tile([128, KY, WM], DT, tag="img")
                    xv = x_d[b, c].rearrange("(a p) w -> p a w", p=128)
                    nc.sync.dma_start(img_t[:, 0:2, :], xv[:, 0:2, :])
                    nc.sync.dma_start(img_t[:, 2:4, :], xv[:, 2:4, :])
                    tqs = []
                    for q in range(KY):
                        p1 = ps1.tile([128, NP], F32, tag="ps1")
                        for k in range(KY):
                            nc.tensor.matmul(
                                p1[:, :],
                                img_t[:, k, q * 128 : (q + 1) * 128],
                                ayt_t[:, k, :],
                                start=(k == 0),
                                stop=(k == KY - 1),
                            )
                        tq = tq_pool.tile([128, SIZE], DT, tag="tq")
                        nc.vector.tensor_copy(tq[:], p1[:, :SIZE])
                        tqs.append(tq)
                    tmeta = None
                    if use_meta:
                        pm = ps1.tile([128, NP], F32, tag="ps1")
                        for k in range(KY):
                            nc.tensor.matmul(
                                pm[:1, :],
                                img_t[:, k, HM : HM + 1],
                                ayt_t[:, k, :],
                                start=(k == 0),
                                stop=(k == KY - 1),
                            )
                        tmeta = tq_pool.tile([1, SIZE], DT, tag="tqm")
                        nc.vector.tensor_copy(tmeta[:], pm[:1, :SIZE])
                    pending.append((b, c, tqs, tmeta, axt_t, axt_m))
                    if len(pending) > PIPE_DEPTH:
                        emit_stage2(pending.pop(0))
            for job in pending:
                emit_stage2(job)

    nc.compile()
    return nc


def _get_program(use_meta):
    key = (use_meta, DT_NAME, NP)
    if key not in _prog_cache:
        _prog_cache[key] = _build_program(use_meta)
    return _prog_cache[key]


def _np_dt(name):
    if name == "bfloat16":
        import ml_dtypes

        return ml_dtypes.bfloat16
    if name == "float16":
        return np.float16
    return np.float32


def _round_f32r(a):
    """Pre-round fp32 data to the FP32R grid the PE uses, so device inputs
    are already 'rounded to FP32r' and results are deterministic."""
    if F32R_BITS is None or F32R_BITS >= 23:
        return a
    drop = 23 - F32R_BITS
    u = np.ascontiguousarray(a).view(np.uint32)
    if F32R_MODE == "rne":
        half = np.uint32((1 << (drop - 1)) - 1)
        lsb = (u >> drop) & np.uint32(1)
        u = u + half + lsb
    u = (u >> drop) << drop
    return u.view(np.float32)


# ---------------------------------------------------------------- entry point
def kernel(x, trace=False):
    from concourse.bass_utils import run_bass_kernel_spmd

    x = np.ascontiguousarray(np.asarray(x, dtype=np.float32))
    assert x.shape == (B_FULL, C, HM, WM), x.shape

    ayt, axt = _build_weights(x)
    use_meta = bool(np.abs(axt[:, HM, :]).max() > 0)

    nc = _get_program(use_meta)

    ndt = _np_dt(DT_NAME)
    xc = x.astype(ndt, copy=False)
    aytc = ayt.astype(ndt, copy=False)
    axtc = axt.astype(ndt, copy=False)
    if DT_NAME == "float32r":
        xc = _round_f32r(xc)
        aytc = _round_f32r(aytc)
        axtc = _round_f32r(axtc)

    in_maps = []
    for k in range(N_CORES):
        sl = slice(k * BPC, (k + 1) * BPC)
        in_maps.append(
            {
                "x_s": np.ascontiguousarray(xc[sl]),
                "ayt": np.ascontiguousarray(aytc[sl]),
                "axt": np.ascontiguousarray(axtc[sl]),
            }
        )

    res = run_bass_kernel_spmd(nc, in_maps, list(range(N_CORES)), trace=trace)
    out = np.concatenate([res.results[k]["out"] for k in range(N_CORES)], axis=0)
    if trace:
        kernel.last_exec_ns = res.exec_time_ns
        kernel.last_results = res
    return out


# revision 15
# speedup vs baseline: 2.2226x; 1.0249x over previous
"""Trainium2 Bass kernel for nn_DataAugmentation (RandomResizedCrop + hflip batch).

Strategy
--------
Data parallel over batch: core k handles samples [8k, 8k+8).

All data-dependent work (RNG replication, crop params, bilinear weights,
hflip column remap) is tiny and happens on host; it is encoded into two
small per-sample interpolation matrices so the device does only dense
matmuls over its full input shard:

    out[b, c] = (A_yT[b].T @ img[b, c]) @ A_xT[b]
              = stage1 (row interp)      stage2 (col interp)

with img the ORIGINAL image (incl. metadata col, 512x513); the horizontal
flip is absorbed into A_xT's source-column index map.

Device stage 1 computes t.T = img.T @ A_yT chunk-wise (lhsT = img slice —
natural row-major layout, no transposes anywhere), stage 2 computes
out = (t.T).T @ A_xT.
"""

import numpy as np

SIZE = 224
HM = 512
WM = 513  # includes metadata column
B_FULL = 64
C = 3
N_CORES = 8
BPC = B_FULL // N_CORES  # samples per core
KY = HM // 128  # k-tiles over rows / cols

# dtype config: "float32" (exact), "float16", "bfloat16", "float32r"
DT_NAME = "float16"
NP = 224  # padded free dim (>=224; must be 256 for float32r full-rate matmuls)
F32R_BITS = 11  # HW-probed: FP32R keeps 11 mantissa bits, round-nearest-even
F32R_MODE = "rne"

SCALE = (0.1, 1.0)
RATIO = (0.8, 1.25)
N_TRIES = 10

_prog_cache = {}


# ---------------------------------------------------------------- host math
def _compute_params(x):
    """Replicates reference._get_params + flip RNG exactly (jax on CPU)."""
    import jax
    import jax.numpy as jnp

    cpu = jax.devices("cpu")[0]
    with jax.default_device(cpu):
        B = x.shape[0]
        H = x[:, 0, 0, -1].astype(np.int32)
        W = x[:, 1, 0, -1].astype(np.int32)
        key = jax.random.key(42)
        kflip, kparams = jax.random.split(key)
        flip_mask = np.asarray(jax.random.uniform(kflip, (B,)) > 0.5)

        Hf = jnp.asarray(H, jnp.float32)
        Wf = jnp.asarray(W, jnp.float32)
        area = Hf * Wf
        log_lo = np.log(RATIO[0]).astype(np.float32)
        log_hi = np.log(RATIO[1]).astype(np.float32)
        final_h = jnp.full((B,), -1.0, dtype=jnp.float32)
        final_w = jnp.full((B,), -1.0, dtype=jnp.float32)
        final_i = jnp.full((B,), -1.0, dtype=jnp.float32)
        final_j = jnp.full((B,), -1.0, dtype=jnp.float32)
        success = jnp.zeros((B,), dtype=bool)
        for t in range(N_TRIES):
            k1, k2, k3, k4 = jax.random.split(jax.random.fold_in(kparams, t), 4)
            target_area = area * jax.random.uniform(
                k1, (B,), minval=SCALE[0], maxval=SCALE[1]
            )
            aspect = jnp.exp(jax.random.uniform(k2, (B,), minval=log_lo, maxval=log_hi))
            crop_w = jnp.round(jnp.sqrt(target_area * aspect))
            crop_h = jnp.round(jnp.sqrt(target_area / aspect))
            valid = (
                (crop_w > 0) & (crop_w <= Wf) & (crop_h > 0) & (crop_h <= Hf) & (~success)
            )
            max_i = jnp.clip(Hf - crop_h + 1.0, 1.0, None)
            max_j = jnp.clip(Wf - crop_w + 1.0, 1.0, None)
            rand_i = jnp.floor(jax.random.uniform(k3, (B,)) * max_i)
            rand_j = jnp.floor(jax.random.uniform(k4, (B,)) * max_j)
            final_h = jnp.where(valid, crop_h, final_h)
            final_w = jnp.where(valid, crop_w, final_w)
            final_i = jnp.where(valid, rand_i, final_i)
            final_j = jnp.where(valid, rand_j, final_j)
            success = success | valid
        failed = ~success
        in_ratio = Wf / Hf
        fb_w = jnp.where(in_ratio > RATIO[1], jnp.round(Hf * RATIO[1]), Wf)
        fb_h = jnp.where(in_ratio < RATIO[0], jnp.round(Wf / RATIO[0]), Hf)
        fb_i = jnp.floor((Hf - fb_h) / 2.0)
        fb_j = jnp.floor((Wf - fb_w) / 2.0)
        final_h = jnp.where(failed, fb_h, final_h)
        final_w = jnp.where(failed, fb_w, final_w)
        final_i = jnp.where(failed, fb_i, final_i)
        final_j = jnp.where(failed, fb_j, final_j)
        i = np.asarray(final_i.astype(jnp.int32))
        j = np.asarray(final_j.astype(jnp.int32))
        h = np.asarray(final_h.astype(jnp.int32))
        w = np.asarray(final_w.astype(jnp.int32))
    return flip_mask, i, j, h, w


def _axis_weights(start, length, n_src_max):
    lf = np.float32(length)
    s = (np.arange(SIZE, dtype=np.float32) + np.float32(0.5)) * lf / np.float32(
        SIZE
    ) - np.float32(0.5)
    s = np.clip(s, np.float32(0.0), lf - np.float32(1.0))
    s0 = np.floor(s)
    frac = (s - s0).astype(np.float32)
    i0 = np.clip(s0.astype(np.int64) + start, 0, n_src_max - 1)
    hi = int(np.clip(start + length - 1, 0, n_src_max - 1))
    i1 = np.clip(i0 + 1, 0, hi)
    return i0, i1, frac


def _build_weights(x):
    """A_yT [B, 512, NP], A_xT [B, 513, NP] f32 (flip folded into A_xT)."""
    Bn = x.shape[0]
    flip_mask, i, j, h, w = _compute_params(x)
    ayt = np.zeros((Bn, HM, NP), dtype=np.float32)
    axt = np.zeros((Bn, WM, NP), dtype=np.float32)
    cols = np.arange(SIZE)
    for b in range(Bn):
        y0, y1, wy = _axis_weights(int(i[b]), int(h[b]), HM)
        np.add.at(ayt[b], (y0, cols), (1.0 - wy))
        np.add.at(ayt[b], (y1, cols), wy)
        x0, x1, wx = _axis_weights(int(j[b]), int(w[b]), HM)
        if flip_mask[b]:
            x0 = HM - x0
            x1 = HM - x1
        np.add.at(axt[b], (x0, cols), (1.0 - wx))
        np.add.at(axt[b], (x1, cols), wx)
    return ayt, axt


# ---------------------------------------------------------------- device prog
def _build_program(use_meta):
    import concourse.mybir as mybir
    import concourse.tile as tile
    from concourse import bacc

    DT = getattr(mybir.dt, DT_NAME)
    F32 = mybir.dt.float32

    nc = bacc.Bacc(None)
    x_d = nc.dram_tensor("x_s", [BPC, C, HM, WM], DT, kind="ExternalInput")
    ayt_d = nc.dram_tensor("ayt", [BPC, HM, NP], DT, kind="ExternalInput")
    axt_d = nc.dram_tensor("axt", [BPC, WM, NP], DT, kind="ExternalInput")
    out_d = nc.dram_tensor("out", [BPC, C, SIZE, SIZE], F32, kind="ExternalOutput")

    with tile.TileContext(nc) as tc:
        with (
            tc.tile_pool(name="img", bufs=8) as img_pool,
            tc.tile_pool(name="wy", bufs=4) as wy_pool,
            tc.tile_pool(name="wx", bufs=4) as wx_pool,
            tc.tile_pool(name="tq", bufs=20) as tq_pool,
            tc.tile_pool(name="ob", bufs=8) as out_pool,
            tc.tile_pool(name="ps1", bufs=4, space="PSUM") as ps1,
            tc.tile_pool(name="ps2", bufs=4, space="PSUM") as ps2,
        ):

            def emit_stage2(job):
                b, c, tqs, tmeta, axt_t, axt_m = job
                for m2 in range(2):
                    m2sz = min(128, SIZE - m2 * 128)
                    sl = slice(m2 * 128, m2 * 128 + m2sz)
                    po = ps2.tile([128, NP], F32, tag="ps2")
                    for k2 in range(KY):
                        nc.tensor.matmul(
                            po[:m2sz, :],
                            tqs[k2][:, sl],
                            axt_t[:, k2, :],
                            start=(k2 == 0),
                            stop=(k2 == KY - 1 and not use_meta),
                        )
                    if use_meta:
                        nc.tensor.matmul(
                            po[:m2sz, :],
                            tmeta[:1, sl],
                            axt_m[:1, :],
                            start=False,
                            stop=True,
                        )
                    ob = out_pool.tile([128, SIZE], F32, tag="ob")
                    nc.scalar.copy(ob[:m2sz, :], po[:m2sz, :SIZE])
                    nc.sync.dma_start(out_d[b, c, sl, :], ob[:m2sz, :])

            pending = []
            PIPE_DEPTH = 2
            for b in range(BPC):
                ayt_t = wy_pool.tile([128, KY, NP], DT, tag="wy")
                nc.sync.dma_start(
                    ayt_t[:], ayt_d[b].rearrange("(a p) n -> p a n", p=128)
                )
                axt_t = wx_pool.tile([128, KY, NP], DT, tag="wx")
                nc.sync.dma_start(
                    axt_t[:], axt_d[b, :HM].rearrange("(a p) n -> p a n", p=128)
                )
                axt_m = None
                if use_meta:
                    axt_m = wx_pool.tile([1, NP], DT, tag="wxm")
                    nc.sync.dma_start(axt_m[:], axt_d[b, HM : HM + 1, :])
                for c in range(C):
                    img_t = img_pool.tile([128, KY, WM], DT, tag="img")
                    xv = x_d[b, c].rearrange("(a p) w -> p a w", p=128)
                    nc.sync.dma_start(img_t[:, 0:2, :], xv[:, 0:2, :])
                    nc.sync.dma_start(img_t[:, 2:4, :], xv[:, 2:4, :])
                    tqs = []
                    for q in range(KY):
                        p1 = ps1.tile([128, NP], F32, tag="ps1")
                        for k in range(KY):
                            nc.tensor.matmul(
                                p1[:, :],
                                img_t[:, k, q * 128 : (q + 1) * 128],
                                ayt_t[:, k, :],
                                start=(k == 0),
                                stop=(k == KY - 1),
                            )
                        tq = tq_pool.tile([128, SIZE], DT, tag="tq")
                        nc.vector.tensor_copy(tq[:], p1[:, :SIZE])
                        tqs.append(tq)
                    tmeta = None
                    if use_meta:
                        pm = ps1.tile([128, NP], F32, tag="ps1")
                        for k in range(KY):
                            nc.tensor.matmul(
                                pm[:1, :],
                                img_t[:, k, HM : HM + 1],
                                ayt_t[:, k, :],
                                start=(k == 0),
                                stop=(k == KY - 1),
                            )
                        tmeta = tq_pool.tile([1, SIZE], DT, tag="tqm")
                        nc.vector.tensor_copy(tmeta[:], pm[:1, :SIZE])
                    pending.append((b, c, tqs, tmeta, axt_t, axt_m))
                    if len(pending) > PIPE_DEPTH:
                        emit_stage2(pending.pop(0))
            for job in pending:
                emit_stage2(job)

    nc.compile()
    return nc


def _get_program(use_meta):
    key = (use_meta, DT_NAME, NP)
    if key not in _prog_cache:
        _prog_cache[key] = _build_program(use_meta)
    return _prog_cache[key]


def _np_dt(name):
    if name == "bfloat16":
        import ml_dtypes

        return ml_dtypes.bfloat16
    if name == "float16":
        return np.float16
    return np.float32


def _round_f32r(a):
    """Pre-round fp32 data to the FP32R grid the PE uses, so device inputs
    are already 'rounded to FP32r' and results are deterministic."""
    if F32R_BITS is None or F32R_BITS >= 23:
        return a
    drop = 23 - F32R_BITS
    u = np.ascontiguousarray(a).view(np.uint32)
    if F32R_MODE == "rne":
        half = np.uint32((1 << (drop - 1)) - 1)
        lsb = (u >> drop) & np.uint32(1)
        u = u + half + lsb
    u = (u >> drop) << drop
    return u.view(np.float32)


# ---------------------------------------------------------------- entry point
def kernel(x, trace=False):
    from concourse.bass_utils import run_bass_kernel_spmd

    x = np.ascontiguousarray(np.asarray(x, dtype=np.float32))
    assert x.shape == (B_FULL, C, HM, WM), x.shape

    ayt, axt = _build_weights(x)
    use_meta = bool(np.abs(axt[:, HM, :]).max() > 0)
    import os

    if os.environ.get("KERNEL_FORCE_USE_META"):
        use_meta = True

    nc = _get_program(use_meta)

    ndt = _np_dt(DT_NAME)
    xc = x.astype(ndt, copy=False)
    aytc = ayt.astype(ndt, copy=False)
    axtc = axt.astype(ndt, copy=False)
    if DT_NAME == "float32r":
        xc = _round_f32r(xc)
        aytc = _round_f32r(aytc)
        axtc = _round_f32r(axtc)

    in_maps = []
    for k in range(N_CORES):
        sl = slice(k * BPC, (k + 1) * BPC)
        in_maps.append(
            {
                "x_s": np.ascontiguousarray(xc[sl]),
                "ayt": np.ascontiguousarray(aytc[sl]),
                "axt": np.ascontiguousarray(axtc[sl]),
            }
        )

    res = run_bass_kernel_spmd(nc, in_maps, list(range(N_CORES)), trace=trace)
    out = np.concatenate([res.results[k]["out"] for k in range(N_CORES)], axis=0)
    if trace:
        kernel.last_exec_ns = res.exec_time_ns
        kernel.last_results = res
    return out


# revision 18
# speedup vs baseline: 2.4628x; 1.1081x over previous
"""Trainium2 Bass kernel for nn_DataAugmentation (RandomResizedCrop + hflip batch).

Strategy
--------
Data parallel over batch: core k handles samples [8k, 8k+8).

All data-dependent work (RNG replication, crop params, bilinear weights,
hflip column remap) is tiny and happens on host; it is encoded into two
small per-sample interpolation matrices so the device does only dense
matmuls over its full input shard:

    out[b, c] = (A_yT[b].T @ img[b, c]) @ A_xT[b]
              = stage1 (row interp)      stage2 (col interp)

with img the ORIGINAL image (incl. metadata col, 512x513); the horizontal
flip is absorbed into A_xT's source-column index map.

Device stage 1 computes t.T = img.T @ A_yT chunk-wise (lhsT = img slice —
natural row-major layout, no transposes anywhere), stage 2 computes
out = (t.T).T @ A_xT.
"""

import numpy as np

SIZE = 224
HM = 512
WM = 513  # includes metadata column
B_FULL = 64
C = 3
N_CORES = 8
BPC = B_FULL // N_CORES  # samples per core
KY = HM // 128  # k-tiles over rows / cols

# dtype config: "float32" (exact), "float16", "bfloat16", "float32r"
DT_NAME = "float16"
NP = 224  # padded free dim (>=224; must be 256 for float32r full-rate matmuls)
F32R_BITS = 11  # HW-probed: FP32R keeps 11 mantissa bits, round-nearest-even
F32R_MODE = "rne"

SCALE = (0.1, 1.0)
RATIO = (0.8, 1.25)
N_TRIES = 10

_prog_cache = {}


# ---------------------------------------------------------------- host math
def _compute_params(x):
    """Replicates reference._get_params + flip RNG exactly (jax on CPU)."""
    import jax
    import jax.numpy as jnp

    cpu = jax.devices("cpu")[0]
    with jax.default_device(cpu):
        B = x.shape[0]
        H = x[:, 0, 0, -1].astype(np.int32)
        W = x[:, 1, 0, -1].astype(np.int32)
        key = jax.random.key(42)
        kflip, kparams = jax.random.split(key)
        flip_mask = np.asarray(jax.random.uniform(kflip, (B,)) > 0.5)

        Hf = jnp.asarray(H, jnp.float32)
        Wf = jnp.asarray(W, jnp.float32)
        area = Hf * Wf
        log_lo = np.log(RATIO[0]).astype(np.float32)
        log_hi = np.log(RATIO[1]).astype(np.float32)
        final_h = jnp.full((B,), -1.0, dtype=jnp.float32)
        final_w = jnp.full((B,), -1.0, dtype=jnp.float32)
        final_i = jnp.full((B,), -1.0, dtype=jnp.float32)
        final_j = jnp.full((B,), -1.0, dtype=jnp.float32)
        success = jnp.zeros((B,), dtype=bool)
        for t in range(N_TRIES):
            k1, k2, k3, k4 = jax.random.split(jax.random.fold_in(kparams, t), 4)
            target_area = area * jax.random.uniform(
                k1, (B,), minval=SCALE[0], maxval=SCALE[1]
            )
            aspect = jnp.exp(jax.random.uniform(k2, (B,), minval=log_lo, maxval=log_hi))
            crop_w = jnp.round(jnp.sqrt(target_area * aspect))
            crop_h = jnp.round(jnp.sqrt(target_area / aspect))
            valid = (
                (crop_w > 0) & (crop_w <= Wf) & (crop_h > 0) & (crop_h <= Hf) & (~success)
            )
            max_i = jnp.clip(Hf - crop_h + 1.0, 1.0, None)
            max_j = jnp.clip(Wf - crop_w + 1.0, 1.0, None)
            rand_i = jnp.floor(jax.random.uniform(k3, (B,)) * max_i)
            rand_j = jnp.floor(jax.random.uniform(k4, (B,)) * max_j)
            final_h = jnp.where(valid, crop_h, final_h)
            final_w = jnp.where(valid, crop_w, final_w)
            final_i = jnp.where(valid, rand_i, final_i)
            final_j = jnp.where(valid, rand_j, final_j)
            success = success | valid
        failed = ~success
        in_ratio = Wf / Hf
        fb_w = jnp.where(in_ratio > RATIO[1], jnp.round(Hf * RATIO[1]), Wf)
        fb_h = jnp.where(in_ratio < RATIO[0], jnp.round(Wf / RATIO[0]), Hf)
        fb_i = jnp.floor((Hf - fb_h) / 2.0)
        fb_j = jnp.floor((Wf - fb_w) / 2.0)
        final_h = jnp.where(failed, fb_h, final_h)
        final_w = jnp.where(failed, fb_w, final_w)
        final_i = jnp.where(failed, fb_i, final_i)
        final_j = jnp.where(failed, fb_j, final_j)
        i = np.asarray(final_i.astype(jnp.int32))
        j = np.asarray(final_j.astype(jnp.int32))
        h = np.asarray(final_h.astype(jnp.int32))
        w = np.asarray(final_w.astype(jnp.int32))
    return flip_mask, i, j, h, w


def _axis_weights(start, length, n_src_max):
    lf = np.float32(length)
    s = (np.arange(SIZE, dtype=np.float32) + np.float32(0.5)) * lf / np.float32(
        SIZE
    ) - np.float32(0.5)
    s = np.clip(s, np.float32(0.0), lf - np.float32(1.0))
    s0 = np.floor(s)
    frac = (s - s0).astype(np.float32)
    i0 = np.clip(s0.astype(np.int64) + start, 0, n_src_max - 1)
    hi = int(np.clip(start + length - 1, 0, n_src_max - 1))
    i1 = np.clip(i0 + 1, 0, hi)
    return i0, i1, frac


def _build_weights(x):
    """A_yT [B, 512, NP], A_xT [B, 513, NP] f32 (flip folded into A_xT)."""
    Bn = x.shape[0]
    flip_mask, i, j, h, w = _compute_params(x)
    ayt = np.zeros((Bn, HM, NP), dtype=np.float32)
    axt = np.zeros((Bn, WM, NP), dtype=np.float32)
    cols = np.arange(SIZE)
    for b in range(Bn):
        y0, y1, wy = _axis_weights(int(i[b]), int(h[b]), HM)
        np.add.at(ayt[b], (y0, cols), (1.0 - wy))
        np.add.at(ayt[b], (y1, cols), wy)
        x0, x1, wx = _axis_weights(int(j[b]), int(w[b]), HM)
        if flip_mask[b]:
            x0 = HM - x0
            x1 = HM - x1
        np.add.at(axt[b], (x0, cols), (1.0 - wx))
        np.add.at(axt[b], (x1, cols), wx)
    return ayt, axt


# ---------------------------------------------------------------- device prog
def _build_program(use_meta):
    import concourse.mybir as mybir
    import concourse.tile as tile
    from concourse import bacc

    DT = getattr(mybir.dt, DT_NAME)
    F32 = mybir.dt.float32

    nc = bacc.Bacc(None)
    x_d = nc.dram_tensor("x_s", [BPC, C, HM, WM], DT, kind="ExternalInput")
    ayt_d = nc.dram_tensor("ayt", [BPC, HM, NP], DT, kind="ExternalInput")
    axt_d = nc.dram_tensor("axt", [BPC, WM, NP], DT, kind="ExternalInput")
    out_d = nc.dram_tensor("out", [BPC, C, SIZE, SIZE], F32, kind="ExternalOutput")

    with tile.TileContext(nc) as tc:
        with (
            tc.tile_pool(name="img", bufs=8) as img_pool,
            tc.tile_pool(name="wy", bufs=4) as wy_pool,
            tc.tile_pool(name="wx", bufs=4) as wx_pool,
            tc.tile_pool(name="tq", bufs=20) as tq_pool,
            tc.tile_pool(name="ob", bufs=8) as out_pool,
            tc.tile_pool(name="ps1", bufs=4, space="PSUM") as ps1,
            tc.tile_pool(name="ps2", bufs=4, space="PSUM") as ps2,
        ):

            def emit_stage2(job):
                b, c, tqs, tmeta, axt_t, axt_m = job
                for m2 in range(2):
                    m2sz = min(128, SIZE - m2 * 128)
                    sl = slice(m2 * 128, m2 * 128 + m2sz)
                    po = ps2.tile([128, NP], F32, tag="ps2")
                    for k2 in range(KY):
                        nc.tensor.matmul(
                            po[:m2sz, :],
                            tqs[k2][:, sl],
                            axt_t[:, k2, :],
                            start=(k2 == 0),
                            stop=(k2 == KY - 1 and not use_meta),
                        )
                    if use_meta:
                        nc.tensor.matmul(
                            po[:m2sz, :],
                            tmeta[:1, sl],
                            axt_m[:1, :],
                            start=False,
                            stop=True,
                        )
                    ob = out_pool.tile([128, SIZE], F32, tag="ob")
                    nc.scalar.copy(ob[:m2sz, :], po[:m2sz, :SIZE])
                    nc.sync.dma_start(out_d[b, c, sl, :], ob[:m2sz, :])

            pending = []
            PIPE_DEPTH = 3
            for b in range(BPC):
                # Row-interleaved k-tiles: partition p of k-tile a holds source
                # row 4p+a, so each partition's DMA run is 4 consecutive rows
                # (contiguous in HBM). img below uses the same mapping, so the
                # matmul contraction pairs identical rows — order is irrelevant.
                ayt_t = wy_pool.tile([128, KY, NP], DT, tag="wy")
                nc.sync.dma_start(
                    ayt_t[:], ayt_d[b].rearrange("(p a) n -> p a n", p=128)
                )
                axt_t = wx_pool.tile([128, KY, NP], DT, tag="wx")
                nc.sync.dma_start(
                    axt_t[:], axt_d[b, :HM].rearrange("(a p) n -> p a n", p=128)
                )
                axt_m = None
                if use_meta:
                    axt_m = wx_pool.tile([1, NP], DT, tag="wxm")
                    nc.sync.dma_start(axt_m[:], axt_d[b, HM : HM + 1, :])
                for c in range(C):
                    img_t = img_pool.tile([128, KY, WM], DT, tag="img")
                    xv = x_d[b, c].rearrange("(p a) w -> p a w", p=128)
                    nc.sync.dma_start(img_t[:, 0:2, :], xv[:, 0:2, :])
                    nc.sync.dma_start(img_t[:, 2:4, :], xv[:, 2:4, :])
                    tqs = []
                    for q in range(KY):
                        p1 = ps1.tile([128, NP], F32, tag="ps1")
                        for k in range(KY):
                            nc.tensor.matmul(
                                p1[:, :],
                                img_t[:, k, q * 128 : (q + 1) * 128],
                                ayt_t[:, k, :],
                                start=(k == 0),
                                stop=(k == KY - 1),
                            )
                        tq = tq_pool.tile([128, SIZE], DT, tag="tq")
                        nc.vector.tensor_copy(tq[:], p1[:, :SIZE])
                        tqs.append(tq)
                    tmeta = None
                    if use_meta:
                        pm = ps1.tile([128, NP], F32, tag="ps1")
                        for k in range(KY):
                            nc.tensor.matmul(
                                pm[:1, :],
                                img_t[:, k, HM : HM + 1],
                                ayt_t[:, k, :],
                                start=(k == 0),
                                stop=(k == KY - 1),
                            )
                        tmeta = tq_pool.tile([1, SIZE], DT, tag="tqm")
                        nc.vector.tensor_copy(tmeta[:], pm[:1, :SIZE])
                    pending.append((b, c, tqs, tmeta, axt_t, axt_m))
                    if len(pending) > PIPE_DEPTH:
                        emit_stage2(pending.pop(0))
            for job in pending:
                emit_stage2(job)

    nc.compile()
    return nc


def _get_program(use_meta):
    key = (use_meta, DT_NAME, NP)
    if key not in _prog_cache:
        _prog_cache[key] = _build_program(use_meta)
    return _prog_cache[key]


def _np_dt(name):
    if name == "bfloat16":
        import ml_dtypes

        return ml_dtypes.bfloat16
    if name == "float16":
        return np.float16
    return np.float32


def _round_f32r(a):
    """Pre-round fp32 data to the FP32R grid the PE uses, so device inputs
    are already 'rounded to FP32r' and results are deterministic."""
    if F32R_BITS is None or F32R_BITS >= 23:
        return a
    drop = 23 - F32R_BITS
    u = np.ascontiguousarray(a).view(np.uint32)
    if F32R_MODE == "rne":
        half = np.uint32((1 << (drop - 1)) - 1)
        lsb = (u >> drop) & np.uint32(1)
        u = u + half + lsb
    u = (u >> drop) << drop
    return u.view(np.float32)


# ---------------------------------------------------------------- entry point
def kernel(x, trace=False):
    from concourse.bass_utils import run_bass_kernel_spmd

    x = np.ascontiguousarray(np.asarray(x, dtype=np.float32))
    assert x.shape == (B_FULL, C, HM, WM), x.shape

    ayt, axt = _build_weights(x)
    use_meta = bool(np.abs(axt[:, HM, :]).max() > 0)
    import os

    if os.environ.get("KERNEL_FORCE_USE_META"):
        use_meta = True

    nc = _get_program(use_meta)

    ndt = _np_dt(DT_NAME)
    xc = x.astype(ndt, copy=False)
    aytc = ayt.astype(ndt, copy=False)
    axtc = axt.astype(ndt, copy=False)
    if DT_NAME == "float32r":
        xc = _round_f32r(xc)
        aytc = _round_f32r(aytc)
        axtc = _round_f32r(axtc)

    in_maps = []
    for k in range(N_CORES):
        sl = slice(k * BPC, (k + 1) * BPC)
        in_maps.append(
            {
                "x_s": np.ascontiguousarray(xc[sl]),
                "ayt": np.ascontiguousarray(aytc[sl]),
                "axt": np.ascontiguousarray(axtc[sl]),
            }
        )

    res = run_bass_kernel_spmd(nc, in_maps, list(range(N_CORES)), trace=trace)
    out = np.concatenate([res.results[k]["out"] for k in range(N_CORES)], axis=0)
    if trace:
        kernel.last_exec_ns = res.exec_time_ns
        kernel.last_results = res
    return out


# revision 20
# speedup vs baseline: 2.4904x; 1.0112x over previous
"""Trainium2 Bass kernel for nn_DataAugmentation (RandomResizedCrop + hflip batch).

Strategy
--------
Data parallel over batch: core k handles samples [8k, 8k+8).

All data-dependent work (RNG replication, crop params, bilinear weights,
hflip column remap) is tiny and happens on host; it is encoded into two
small per-sample interpolation matrices so the device does only dense
matmuls over its full input shard:

    out[b, c] = (A_yT[b].T @ img[b, c]) @ A_xT[b]
              = stage1 (row interp)      stage2 (col interp)

with img the ORIGINAL image (incl. metadata col, 512x513); the horizontal
flip is absorbed into A_xT's source-column index map.

Device stage 1 computes t.T = img.T @ A_yT chunk-wise (lhsT = img slice —
natural row-major layout, no transposes anywhere), stage 2 computes
out = (t.T).T @ A_xT.
"""

import numpy as np

SIZE = 224
HM = 512
WM = 513  # includes metadata column
B_FULL = 64
C = 3
N_CORES = 8
BPC = B_FULL // N_CORES  # samples per core
KY = HM // 128  # k-tiles over rows / cols

# dtype config: "float32" (exact), "float16", "bfloat16", "float32r"
DT_NAME = "float16"
NP = 224  # padded free dim (>=224; must be 256 for float32r full-rate matmuls)
F32R_BITS = 11  # HW-probed: FP32R keeps 11 mantissa bits, round-nearest-even
F32R_MODE = "rne"

SCALE = (0.1, 1.0)
RATIO = (0.8, 1.25)
N_TRIES = 10

_prog_cache = {}


# ---------------------------------------------------------------- host math
def _compute_params(x):
    """Replicates reference._get_params + flip RNG exactly (jax on CPU)."""
    import jax
    import jax.numpy as jnp

    cpu = jax.devices("cpu")[0]
    with jax.default_device(cpu):
        B = x.shape[0]
        H = x[:, 0, 0, -1].astype(np.int32)
        W = x[:, 1, 0, -1].astype(np.int32)
        key = jax.random.key(42)
        kflip, kparams = jax.random.split(key)
        flip_mask = np.asarray(jax.random.uniform(kflip, (B,)) > 0.5)

        Hf = jnp.asarray(H, jnp.float32)
        Wf = jnp.asarray(W, jnp.float32)
        area = Hf * Wf
        log_lo = np.log(RATIO[0]).astype(np.float32)
        log_hi = np.log(RATIO[1]).astype(np.float32)
        final_h = jnp.full((B,), -1.0, dtype=jnp.float32)
        final_w = jnp.full((B,), -1.0, dtype=jnp.float32)
        final_i = jnp.full((B,), -1.0, dtype=jnp.float32)
        final_j = jnp.full((B,), -1.0, dtype=jnp.float32)
        success = jnp.zeros((B,), dtype=bool)
        for t in range(N_TRIES):
            k1, k2, k3, k4 = jax.random.split(jax.random.fold_in(kparams, t), 4)
            target_area = area * jax.random.uniform(
                k1, (B,), minval=SCALE[0], maxval=SCALE[1]
            )
            aspect = jnp.exp(jax.random.uniform(k2, (B,), minval=log_lo, maxval=log_hi))
            crop_w = jnp.round(jnp.sqrt(target_area * aspect))
            crop_h = jnp.round(jnp.sqrt(target_area / aspect))
            valid = (
                (crop_w > 0) & (crop_w <= Wf) & (crop_h > 0) & (crop_h <= Hf) & (~success)
            )
            max_i = jnp.clip(Hf - crop_h + 1.0, 1.0, None)
            max_j = jnp.clip(Wf - crop_w + 1.0, 1.0, None)
            rand_i = jnp.floor(jax.random.uniform(k3, (B,)) * max_i)
            rand_j = jnp.floor(jax.random.uniform(k4, (B,)) * max_j)
            final_h = jnp.where(valid, crop_h, final_h)
            final_w = jnp.where(valid, crop_w, final_w)
            final_i = jnp.where(valid, rand_i, final_i)
            final_j = jnp.where(valid, rand_j, final_j)
            success = success | valid
        failed = ~success
        in_ratio = Wf / Hf
        fb_w = jnp.where(in_ratio > RATIO[1], jnp.round(Hf * RATIO[1]), Wf)
        fb_h = jnp.where(in_ratio < RATIO[0], jnp.round(Wf / RATIO[0]), Hf)
        fb_i = jnp.floor((Hf - fb_h) / 2.0)
        fb_j = jnp.floor((Wf - fb_w) / 2.0)
        final_h = jnp.where(failed, fb_h, final_h)
        final_w = jnp.where(failed, fb_w, final_w)
        final_i = jnp.where(failed, fb_i, final_i)
        final_j = jnp.where(failed, fb_j, final_j)
        i = np.asarray(final_i.astype(jnp.int32))
        j = np.asarray(final_j.astype(jnp.int32))
        h = np.asarray(final_h.astype(jnp.int32))
        w = np.asarray(final_w.astype(jnp.int32))
    return flip_mask, i, j, h, w


def _axis_weights(start, length, n_src_max):
    lf = np.float32(length)
    s = (np.arange(SIZE, dtype=np.float32) + np.float32(0.5)) * lf / np.float32(
        SIZE
    ) - np.float32(0.5)
    s = np.clip(s, np.float32(0.0), lf - np.float32(1.0))
    s0 = np.floor(s)
    frac = (s - s0).astype(np.float32)
    i0 = np.clip(s0.astype(np.int64) + start, 0, n_src_max - 1)
    hi = int(np.clip(start + length - 1, 0, n_src_max - 1))
    i1 = np.clip(i0 + 1, 0, hi)
    return i0, i1, frac


def _build_weights(x):
    """A_yT [B, 512, NP], A_xT [B, 513, NP] f32 (flip folded into A_xT)."""
    Bn = x.shape[0]
    flip_mask, i, j, h, w = _compute_params(x)
    ayt = np.zeros((Bn, HM, NP), dtype=np.float32)
    axt = np.zeros((Bn, WM, NP), dtype=np.float32)
    cols = np.arange(SIZE)
    for b in range(Bn):
        y0, y1, wy = _axis_weights(int(i[b]), int(h[b]), HM)
        np.add.at(ayt[b], (y0, cols), (1.0 - wy))
        np.add.at(ayt[b], (y1, cols), wy)
        x0, x1, wx = _axis_weights(int(j[b]), int(w[b]), HM)
        if flip_mask[b]:
            x0 = HM - x0
            x1 = HM - x1
        np.add.at(axt[b], (x0, cols), (1.0 - wx))
        np.add.at(axt[b], (x1, cols), wx)
    return ayt, axt


# ---------------------------------------------------------------- device prog
def _build_program(use_meta):
    import concourse.mybir as mybir
    import concourse.tile as tile
    from concourse import bacc

    DT = getattr(mybir.dt, DT_NAME)
    F32 = mybir.dt.float32

    nc = bacc.Bacc(None)
    x_d = nc.dram_tensor("x_s", [BPC, C, HM, WM], DT, kind="ExternalInput")
    ayt_d = nc.dram_tensor("ayt", [BPC, HM, NP], DT, kind="ExternalInput")
    axt_d = nc.dram_tensor("axt", [BPC, WM, NP], DT, kind="ExternalInput")
    out_d = nc.dram_tensor("out", [BPC, C, SIZE, SIZE], F32, kind="ExternalOutput")

    with tile.TileContext(nc) as tc:
        with (
            tc.tile_pool(name="img", bufs=10) as img_pool,
            tc.tile_pool(name="wy", bufs=5) as wy_pool,
            tc.tile_pool(name="wx", bufs=5) as wx_pool,
            tc.tile_pool(name="tq", bufs=24) as tq_pool,
            tc.tile_pool(name="ob", bufs=10) as out_pool,
            tc.tile_pool(name="ps1", bufs=4, space="PSUM") as ps1,
            tc.tile_pool(name="ps2", bufs=4, space="PSUM") as ps2,
        ):

            def emit_stage2(job):
                b, c, tqs, tmeta, axt_t, axt_m = job
                for m2 in range(2):
                    m2sz = min(128, SIZE - m2 * 128)
                    sl = slice(m2 * 128, m2 * 128 + m2sz)
                    po = ps2.tile([128, NP], F32, tag="ps2")
                    for k2 in range(KY):
                        nc.tensor.matmul(
                            po[:m2sz, :],
                            tqs[k2][:, sl],
                            axt_t[:, k2, :],
                            start=(k2 == 0),
                            stop=(k2 == KY - 1 and not use_meta),
                        )
                    if use_meta:
                        nc.tensor.matmul(
                            po[:m2sz, :],
                            tmeta[:1, sl],
                            axt_m[:1, :],
                            start=False,
                            stop=True,
                        )
                    ob = out_pool.tile([128, SIZE], F32, tag="ob")
                    nc.scalar.copy(ob[:m2sz, :], po[:m2sz, :SIZE])
                    nc.sync.dma_start(out_d[b, c, sl, :], ob[:m2sz, :])

            pending = []
            PIPE_DEPTH = 4
            for b in range(BPC):
                # Row-interleaved k-tiles: partition p of k-tile a holds source
                # row 4p+a, so each partition's DMA run is 4 consecutive rows
                # (contiguous in HBM). img below uses the same mapping, so the
                # matmul contraction pairs identical rows — order is irrelevant.
                ayt_t = wy_pool.tile([128, KY, NP], DT, tag="wy")
                nc.sync.dma_start(
                    ayt_t[:], ayt_d[b].rearrange("(p a) n -> p a n", p=128)
                )
                axt_t = wx_pool.tile([128, KY, NP], DT, tag="wx")
                nc.sync.dma_start(
                    axt_t[:], axt_d[b, :HM].rearrange("(a p) n -> p a n", p=128)
                )
                axt_m = None
                if use_meta:
                    axt_m = wx_pool.tile([1, NP], DT, tag="wxm")
                    nc.sync.dma_start(axt_m[:], axt_d[b, HM : HM + 1, :])
                for c in range(C):
                    img_t = img_pool.tile([128, KY, WM], DT, tag="img")
                    xv = x_d[b, c].rearrange("(p a) w -> p a w", p=128)
                    nc.sync.dma_start(img_t[:, 0:2, :], xv[:, 0:2, :])
                    nc.sync.dma_start(img_t[:, 2:4, :], xv[:, 2:4, :])
                    tqs = []
                    for q in range(KY):
                        p1 = ps1.tile([128, NP], F32, tag="ps1")
                        for k in range(KY):
                            nc.tensor.matmul(
                                p1[:, :],
                                img_t[:, k, q * 128 : (q + 1) * 128],
                                ayt_t[:, k, :],
                                start=(k == 0),
                                stop=(k == KY - 1),
                            )
                        tq = tq_pool.tile([128, SIZE], DT, tag="tq")
                        nc.vector.tensor_copy(tq[:], p1[:, :SIZE])
                        tqs.append(tq)
                    tmeta = None
                    if use_meta:
                        pm = ps1.tile([128, NP], F32, tag="ps1")
                        for k in range(KY):
                            nc.tensor.matmul(
                                pm[:1, :],
                                img_t[:, k, HM : HM + 1],
                                ayt_t[:, k, :],
                                start=(k == 0),
                                stop=(k == KY - 1),
                            )
                        tmeta = tq_pool.tile([1, SIZE], DT, tag="tqm")
                        nc.vector.tensor_copy(tmeta[:], pm[:1, :SIZE])
                    pending.append((b, c, tqs, tmeta, axt_t, axt_m))
                    if len(pending) > PIPE_DEPTH:
                        emit_stage2(pending.pop(0))
            for job in pending:
                emit_stage2(job)

    nc.compile()
    return nc


def _get_program(use_meta):
    key = (use_meta, DT_NAME, NP)
    if key not in _prog_cache:
        _prog_cache[key] = _build_program(use_meta)
    return _prog_cache[key]


def _np_dt(name):
    if name == "bfloat16":
        import ml_dtypes

        return ml_dtypes.bfloat16
    if name == "float16":
        return np.float16
    return np.float32


def _round_f32r(a):
    """Pre-round fp32 data to the FP32R grid the PE uses, so device inputs
    are already 'rounded to FP32r' and results are deterministic."""
    if F32R_BITS is None or F32R_BITS >= 23:
        return a
    drop = 23 - F32R_BITS
    u = np.ascontiguousarray(a).view(np.uint32)
    if F32R_MODE == "rne":
        half = np.uint32((1 << (drop - 1)) - 1)
        lsb = (u >> drop) & np.uint32(1)
        u = u + half + lsb
    u = (u >> drop) << drop
    return u.view(np.float32)


# ---------------------------------------------------------------- entry point
def kernel(x, trace=False):
    from concourse.bass_utils import run_bass_kernel_spmd

    x = np.ascontiguousarray(np.asarray(x, dtype=np.float32))
    assert x.shape == (B_FULL, C, HM, WM), x.shape

    ayt, axt = _build_weights(x)
    use_meta = bool(np.abs(axt[:, HM, :]).max() > 0)
    import os

    if os.environ.get("KERNEL_FORCE_USE_META"):
        use_meta = True

    nc = _get_program(use_meta)

    ndt = _np_dt(DT_NAME)
    xc = x.astype(ndt, copy=False)
    aytc = ayt.astype(ndt, copy=False)
    axtc = axt.astype(ndt, copy=False)
    if DT_NAME == "float32r":
        xc = _round_f32r(xc)
        aytc = _round_f32r(aytc)
        axtc = _round_f32r(axtc)

    in_maps = []
    for k in range(N_CORES):
        sl = slice(k * BPC, (k + 1) * BPC)
        in_maps.append(
            {
                "x_s": np.ascontiguousarray(xc[sl]),
                "ayt": np.ascontiguousarray(aytc[sl]),
                "axt": np.ascontiguousarray(axtc[sl]),
            }
        )

    res = run_bass_kernel_spmd(nc, in_maps, list(range(N_CORES)), trace=trace)
    out = np.concatenate([res.results[k]["out"] for k in range(N_CORES)], axis=0)
    if trace:
        kernel.last_exec_ns = res.exec_time_ns
        kernel.last_results = res
    return out


# revision 23
# speedup vs baseline: 2.5205x; 1.0121x over previous
"""Trainium2 Bass kernel for nn_DataAugmentation (RandomResizedCrop + hflip batch).

Strategy
--------
Data parallel over batch: core k handles samples [8k, 8k+8).

All data-dependent work (RNG replication, crop params, bilinear weights,
hflip column remap) is tiny and happens on host; it is encoded into two
small per-sample interpolation matrices so the device does only dense
matmuls over its full input shard:

    out[b, c] = (A_yT[b].T @ img[b, c]) @ A_xT[b]
              = stage1 (row interp)      stage2 (col interp)

with img the ORIGINAL image (incl. metadata col, 512x513); the horizontal
flip is absorbed into A_xT's source-column index map.

Device stage 1 computes t.T = img.T @ A_yT chunk-wise (lhsT = img slice —
natural row-major layout, no transposes anywhere), stage 2 computes
out = (t.T).T @ A_xT.
"""

import numpy as np

SIZE = 224
HM = 512
WM = 513  # includes metadata column
B_FULL = 64
C = 3
N_CORES = 8
BPC = B_FULL // N_CORES  # samples per core
KY = HM // 128  # k-tiles over rows / cols

# dtype config: "float32" (exact), "float16", "bfloat16", "float32r"
DT_NAME = "float16"
NP = 224  # padded free dim (>=224; must be 256 for float32r full-rate matmuls)
F32R_BITS = 11  # HW-probed: FP32R keeps 11 mantissa bits, round-nearest-even
F32R_MODE = "rne"

SCALE = (0.1, 1.0)
RATIO = (0.8, 1.25)
N_TRIES = 10

_prog_cache = {}


# ---------------------------------------------------------------- host math
def _compute_params(x):
    """Replicates reference._get_params + flip RNG exactly (jax on CPU)."""
    import jax
    import jax.numpy as jnp

    cpu = jax.devices("cpu")[0]
    with jax.default_device(cpu):
        B = x.shape[0]
        H = x[:, 0, 0, -1].astype(np.int32)
        W = x[:, 1, 0, -1].astype(np.int32)
        key = jax.random.key(42)
        kflip, kparams = jax.random.split(key)
        flip_mask = np.asarray(jax.random.uniform(kflip, (B,)) > 0.5)

        Hf = jnp.asarray(H, jnp.float32)
        Wf = jnp.asarray(W, jnp.float32)
        area = Hf * Wf
        log_lo = np.log(RATIO[0]).astype(np.float32)
        log_hi = np.log(RATIO[1]).astype(np.float32)
        final_h = jnp.full((B,), -1.0, dtype=jnp.float32)
        final_w = jnp.full((B,), -1.0, dtype=jnp.float32)
        final_i = jnp.full((B,), -1.0, dtype=jnp.float32)
        final_j = jnp.full((B,), -1.0, dtype=jnp.float32)
        success = jnp.zeros((B,), dtype=bool)
        for t in range(N_TRIES):
            k1, k2, k3, k4 = jax.random.split(jax.random.fold_in(kparams, t), 4)
            target_area = area * jax.random.uniform(
                k1, (B,), minval=SCALE[0], maxval=SCALE[1]
            )
            aspect = jnp.exp(jax.random.uniform(k2, (B,), minval=log_lo, maxval=log_hi))
            crop_w = jnp.round(jnp.sqrt(target_area * aspect))
            crop_h = jnp.round(jnp.sqrt(target_area / aspect))
            valid = (
                (crop_w > 0) & (crop_w <= Wf) & (crop_h > 0) & (crop_h <= Hf) & (~success)
            )
            max_i = jnp.clip(Hf - crop_h + 1.0, 1.0, None)
            max_j = jnp.clip(Wf - crop_w + 1.0, 1.0, None)
            rand_i = jnp.floor(jax.random.uniform(k3, (B,)) * max_i)
            rand_j = jnp.floor(jax.random.uniform(k4, (B,)) * max_j)
            final_h = jnp.where(valid, crop_h, final_h)
            final_w = jnp.where(valid, crop_w, final_w)
            final_i = jnp.where(valid, rand_i, final_i)
            final_j = jnp.where(valid, rand_j, final_j)
            success = success | valid
        failed = ~success
        in_ratio = Wf / Hf
        fb_w = jnp.where(in_ratio > RATIO[1], jnp.round(Hf * RATIO[1]), Wf)
        fb_h = jnp.where(in_ratio < RATIO[0], jnp.round(Wf / RATIO[0]), Hf)
        fb_i = jnp.floor((Hf - fb_h) / 2.0)
        fb_j = jnp.floor((Wf - fb_w) / 2.0)
        final_h = jnp.where(failed, fb_h, final_h)
        final_w = jnp.where(failed, fb_w, final_w)
        final_i = jnp.where(failed, fb_i, final_i)
        final_j = jnp.where(failed, fb_j, final_j)
        i = np.asarray(final_i.astype(jnp.int32))
        j = np.asarray(final_j.astype(jnp.int32))
        h = np.asarray(final_h.astype(jnp.int32))
        w = np.asarray(final_w.astype(jnp.int32))
    return flip_mask, i, j, h, w


def _axis_weights(start, length, n_src_max):
    lf = np.float32(length)
    s = (np.arange(SIZE, dtype=np.float32) + np.float32(0.5)) * lf / np.float32(
        SIZE
    ) - np.float32(0.5)
    s = np.clip(s, np.float32(0.0), lf - np.float32(1.0))
    s0 = np.floor(s)
    frac = (s - s0).astype(np.float32)
    i0 = np.clip(s0.astype(np.int64) + start, 0, n_src_max - 1)
    hi = int(np.clip(start + length - 1, 0, n_src_max - 1))
    i1 = np.clip(i0 + 1, 0, hi)
    return i0, i1, frac


def _build_weights(x):
    """A_yT [B, 512, NP], A_xT [B, 513, NP] f32 (flip folded into A_xT)."""
    Bn = x.shape[0]
    flip_mask, i, j, h, w = _compute_params(x)
    ayt = np.zeros((Bn, HM, NP), dtype=np.float32)
    axt = np.zeros((Bn, WM, NP), dtype=np.float32)
    cols = np.arange(SIZE)
    for b in range(Bn):
        y0, y1, wy = _axis_weights(int(i[b]), int(h[b]), HM)
        np.add.at(ayt[b], (y0, cols), (1.0 - wy))
        np.add.at(ayt[b], (y1, cols), wy)
        x0, x1, wx = _axis_weights(int(j[b]), int(w[b]), HM)
        if flip_mask[b]:
            x0 = HM - x0
            x1 = HM - x1
        np.add.at(axt[b], (x0, cols), (1.0 - wx))
        np.add.at(axt[b], (x1, cols), wx)
    return ayt, axt


# ---------------------------------------------------------------- device prog
def _build_program(use_meta):
    import concourse.mybir as mybir
    import concourse.tile as tile
    from concourse import bacc

    DT = getattr(mybir.dt, DT_NAME)
    F32 = mybir.dt.float32

    nc = bacc.Bacc(None)
    x_d = nc.dram_tensor("x_s", [BPC, C, HM, WM], DT, kind="ExternalInput")
    ayt_d = nc.dram_tensor("ayt", [BPC, HM, NP], DT, kind="ExternalInput")
    axt_d = nc.dram_tensor("axt", [BPC, WM, NP], DT, kind="ExternalInput")
    out_d = nc.dram_tensor("out", [BPC, C, SIZE, SIZE], F32, kind="ExternalOutput")

    with tile.TileContext(nc) as tc:
        with (
            tc.tile_pool(name="img", bufs=10) as img_pool,
            tc.tile_pool(name="wy", bufs=5) as wy_pool,
            tc.tile_pool(name="wx", bufs=5) as wx_pool,
            tc.tile_pool(name="tq", bufs=24) as tq_pool,
            tc.tile_pool(name="ob", bufs=10) as out_pool,
            tc.tile_pool(name="ps1", bufs=4, space="PSUM") as ps1,
            tc.tile_pool(name="ps2", bufs=4, space="PSUM") as ps2,
        ):

            def emit_stage2(job):
                b, c, tqs, tmeta, axt_t, axt_m = job
                for m2 in range(2):
                    m2sz = min(128, SIZE - m2 * 128)
                    sl = slice(m2 * 128, m2 * 128 + m2sz)
                    po = ps2.tile([128, NP], F32, tag="ps2")
                    for k2 in range(KY):
                        nc.tensor.matmul(
                            po[:m2sz, :],
                            tqs[k2][:, sl],
                            axt_t[:, k2, :],
                            start=(k2 == 0),
                            stop=(k2 == KY - 1 and not use_meta),
                        )
                    if use_meta:
                        nc.tensor.matmul(
                            po[:m2sz, :],
                            tmeta[:1, sl],
                            axt_m[:1, :],
                            start=False,
                            stop=True,
                        )
                    ob = out_pool.tile([128, SIZE], F32, tag="ob")
                    nc.scalar.copy(ob[:m2sz, :], po[:m2sz, :SIZE])
                    nc.sync.dma_start(out_d[b, c, sl, :], ob[:m2sz, :])

            pending = []
            PIPE_DEPTH = 4
            for b in range(BPC):
                # Row-interleaved k-tiles: partition p of k-tile a holds source
                # row 4p+a, so each partition's DMA run is 4 consecutive rows
                # (contiguous in HBM). img below uses the same mapping, so the
                # matmul contraction pairs identical rows — order is irrelevant.
                ayt_t = wy_pool.tile([128, KY, NP], DT, tag="wy")
                nc.sync.dma_start(
                    ayt_t[:], ayt_d[b].rearrange("(p a) n -> p a n", p=128)
                )
                axt_t = wx_pool.tile([128, KY, NP], DT, tag="wx")
                nc.sync.dma_start(
                    axt_t[:], axt_d[b, :HM].rearrange("(a p) n -> p a n", p=128)
                )
                axt_m = None
                if use_meta:
                    axt_m = wx_pool.tile([1, NP], DT, tag="wxm")
                    nc.sync.dma_start(axt_m[:], axt_d[b, HM : HM + 1, :])
                for c in range(C):
                    # Two separate tiles (not one) so stage-1's k=0,1 matmuls
                    # depend only on the first DMA, not both halves.
                    img_lo = img_pool.tile([128, 2, WM], DT, tag="img_lo")
                    img_hi = img_pool.tile([128, 2, WM], DT, tag="img_hi")
                    xv = x_d[b, c].rearrange("(p a) w -> p a w", p=128)
                    nc.sync.dma_start(img_lo[:], xv[:, 0:2, :])
                    nc.sync.dma_start(img_hi[:], xv[:, 2:4, :])

                    def img_k(k):
                        return img_lo[:, k, :] if k < 2 else img_hi[:, k - 2, :]
                    tqs = []
                    for q in range(KY):
                        p1 = ps1.tile([128, NP], F32, tag="ps1")
                        for k in range(KY):
                            nc.tensor.matmul(
                                p1[:, :],
                                img_k(k)[:, q * 128 : (q + 1) * 128],
                                ayt_t[:, k, :],
                                start=(k == 0),
                                stop=(k == KY - 1),
                            )
                        tq = tq_pool.tile([128, SIZE], DT, tag="tq")
                        nc.vector.tensor_copy(tq[:], p1[:, :SIZE])
                        tqs.append(tq)
                    tmeta = None
                    if use_meta:
                        pm = ps1.tile([128, NP], F32, tag="ps1")
                        for k in range(KY):
                            nc.tensor.matmul(
                                pm[:1, :],
                                img_k(k)[:, HM : HM + 1],
                                ayt_t[:, k, :],
                                start=(k == 0),
                                stop=(k == KY - 1),
                            )
                        tmeta = tq_pool.tile([1, SIZE], DT, tag="tqm")
                        nc.vector.tensor_copy(tmeta[:], pm[:1, :SIZE])
                    pending.append((b, c, tqs, tmeta, axt_t, axt_m))
                    if len(pending) > PIPE_DEPTH:
                        emit_stage2(pending.pop(0))
            for job in pending:
                emit_stage2(job)

    nc.compile()
    return nc


def _get_program(use_meta):
    key = (use_meta, DT_NAME, NP)
    if key not in _prog_cache:
        _prog_cache[key] = _build_program(use_meta)
    return _prog_cache[key]


def _np_dt(name):
    if name == "bfloat16":
        import ml_dtypes

        return ml_dtypes.bfloat16
    if name == "float16":
        return np.float16
    return np.float32


def _round_f32r(a):
    """Pre-round fp32 data to the FP32R grid the PE uses, so device inputs
    are already 'rounded to FP32r' and results are deterministic."""
    if F32R_BITS is None or F32R_BITS >= 23:
        return a
    drop = 23 - F32R_BITS
    u = np.ascontiguousarray(a).view(np.uint32)
    if F32R_MODE == "rne":
        half = np.uint32((1 << (drop - 1)) - 1)
        lsb = (u >> drop) & np.uint32(1)
        u = u + half + lsb
    u = (u >> drop) << drop
    return u.view(np.float32)


# ---------------------------------------------------------------- entry point
def kernel(x, trace=False):
    from concourse.bass_utils import run_bass_kernel_spmd

    x = np.ascontiguousarray(np.asarray(x, dtype=np.float32))
    assert x.shape == (B_FULL, C, HM, WM), x.shape

    ayt, axt = _build_weights(x)
    use_meta = bool(np.abs(axt[:, HM, :]).max() > 0)
    import os

    if os.environ.get("KERNEL_FORCE_USE_META"):
        use_meta = True

    nc = _get_program(use_meta)

    ndt = _np_dt(DT_NAME)
    xc = x.astype(ndt, copy=False)
    aytc = ayt.astype(ndt, copy=False)
    axtc = axt.astype(ndt, copy=False)
    if DT_NAME == "float32r":
        xc = _round_f32r(xc)
        aytc = _round_f32r(aytc)
        axtc = _round_f32r(axtc)

    in_maps = []
    for k in range(N_CORES):
        sl = slice(k * BPC, (k + 1) * BPC)
        in_maps.append(
            {
                "x_s": np.ascontiguousarray(xc[sl]),
                "ayt": np.ascontiguousarray(aytc[sl]),
                "axt": np.ascontiguousarray(axtc[sl]),
            }
        )

    res = run_bass_kernel_spmd(nc, in_maps, list(range(N_CORES)), trace=trace)
    out = np.concatenate([res.results[k]["out"] for k in range(N_CORES)], axis=0)
    if trace:
        kernel.last_exec_ns = res.exec_time_ns
        kernel.last_results = res
    return out


# revision 26
# speedup vs baseline: 2.5415x; 1.0083x over previous
"""Trainium2 Bass kernel for nn_DataAugmentation (RandomResizedCrop + hflip batch).

Strategy
--------
Data parallel over batch: core k handles samples [8k, 8k+8).

All data-dependent work (RNG replication, crop params, bilinear weights,
hflip column remap) is tiny and happens on host; it is encoded into two
small per-sample interpolation matrices so the device does only dense
matmuls over its full input shard:

    out[b, c] = (A_yT[b].T @ img[b, c]) @ A_xT[b]
              = stage1 (row interp)      stage2 (col interp)

with img the ORIGINAL image (incl. metadata col, 512x513); the horizontal
flip is absorbed into A_xT's source-column index map.

Device stage 1 computes t.T = img.T @ A_yT chunk-wise (lhsT = img slice —
natural row-major layout, no transposes anywhere), stage 2 computes
out = (t.T).T @ A_xT.
"""

import numpy as np

SIZE = 224
HM = 512
WM = 513  # includes metadata column
B_FULL = 64
C = 3
N_CORES = 8
BPC = B_FULL // N_CORES  # samples per core
KY = HM // 128  # k-tiles over rows / cols

# dtype config: "float32" (exact), "float16", "bfloat16", "float32r"
DT_NAME = "float16"
NP = 224  # padded free dim (>=224; must be 256 for float32r full-rate matmuls)
F32R_BITS = 11  # HW-probed: FP32R keeps 11 mantissa bits, round-nearest-even
F32R_MODE = "rne"

SCALE = (0.1, 1.0)
RATIO = (0.8, 1.25)
N_TRIES = 10

_prog_cache = {}


# ---------------------------------------------------------------- host math
def _compute_params(x):
    """Replicates reference._get_params + flip RNG exactly (jax on CPU)."""
    import jax
    import jax.numpy as jnp

    cpu = jax.devices("cpu")[0]
    with jax.default_device(cpu):
        B = x.shape[0]
        H = x[:, 0, 0, -1].astype(np.int32)
        W = x[:, 1, 0, -1].astype(np.int32)
        key = jax.random.key(42)
        kflip, kparams = jax.random.split(key)
        flip_mask = np.asarray(jax.random.uniform(kflip, (B,)) > 0.5)

        Hf = jnp.asarray(H, jnp.float32)
        Wf = jnp.asarray(W, jnp.float32)
        area = Hf * Wf
        log_lo = np.log(RATIO[0]).astype(np.float32)
        log_hi = np.log(RATIO[1]).astype(np.float32)
        final_h = jnp.full((B,), -1.0, dtype=jnp.float32)
        final_w = jnp.full((B,), -1.0, dtype=jnp.float32)
        final_i = jnp.full((B,), -1.0, dtype=jnp.float32)
        final_j = jnp.full((B,), -1.0, dtype=jnp.float32)
        success = jnp.zeros((B,), dtype=bool)
        for t in range(N_TRIES):
            k1, k2, k3, k4 = jax.random.split(jax.random.fold_in(kparams, t), 4)
            target_area = area * jax.random.uniform(
                k1, (B,), minval=SCALE[0], maxval=SCALE[1]
            )
            aspect = jnp.exp(jax.random.uniform(k2, (B,), minval=log_lo, maxval=log_hi))
            crop_w = jnp.round(jnp.sqrt(target_area * aspect))
            crop_h = jnp.round(jnp.sqrt(target_area / aspect))
            valid = (
                (crop_w > 0) & (crop_w <= Wf) & (crop_h > 0) & (crop_h <= Hf) & (~success)
            )
            max_i = jnp.clip(Hf - crop_h + 1.0, 1.0, None)
            max_j = jnp.clip(Wf - crop_w + 1.0, 1.0, None)
            rand_i = jnp.floor(jax.random.uniform(k3, (B,)) * max_i)
            rand_j = jnp.floor(jax.random.uniform(k4, (B,)) * max_j)
            final_h = jnp.where(valid, crop_h, final_h)
            final_w = jnp.where(valid, crop_w, final_w)
            final_i = jnp.where(valid, rand_i, final_i)
            final_j = jnp.where(valid, rand_j, final_j)
            success = success | valid
        failed = ~success
        in_ratio = Wf / Hf
        fb_w = jnp.where(in_ratio > RATIO[1], jnp.round(Hf * RATIO[1]), Wf)
        fb_h = jnp.where(in_ratio < RATIO[0], jnp.round(Wf / RATIO[0]), Hf)
        fb_i = jnp.floor((Hf - fb_h) / 2.0)
        fb_j = jnp.floor((Wf - fb_w) / 2.0)
        final_h = jnp.where(failed, fb_h, final_h)
        final_w = jnp.where(failed, fb_w, final_w)
        final_i = jnp.where(failed, fb_i, final_i)
        final_j = jnp.where(failed, fb_j, final_j)
        i = np.asarray(final_i.astype(jnp.int32))
        j = np.asarray(final_j.astype(jnp.int32))
        h = np.asarray(final_h.astype(jnp.int32))
        w = np.asarray(final_w.astype(jnp.int32))
    return flip_mask, i, j, h, w


def _axis_weights(start, length, n_src_max):
    lf = np.float32(length)
    s = (np.arange(SIZE, dtype=np.float32) + np.float32(0.5)) * lf / np.float32(
        SIZE
    ) - np.float32(0.5)
    s = np.clip(s, np.float32(0.0), lf - np.float32(1.0))
    s0 = np.floor(s)
    frac = (s - s0).astype(np.float32)
    i0 = np.clip(s0.astype(np.int64) + start, 0, n_src_max - 1)
    hi = int(np.clip(start + length - 1, 0, n_src_max - 1))
    i1 = np.clip(i0 + 1, 0, hi)
    return i0, i1, frac


def _build_weights(x):
    """A_yT [B, 512, NP], A_xT [B, 513, NP] f32 (flip folded into A_xT)."""
    Bn = x.shape[0]
    flip_mask, i, j, h, w = _compute_params(x)
    ayt = np.zeros((Bn, HM, NP), dtype=np.float32)
    axt = np.zeros((Bn, WM, NP), dtype=np.float32)
    cols = np.arange(SIZE)
    for b in range(Bn):
        y0, y1, wy = _axis_weights(int(i[b]), int(h[b]), HM)
        np.add.at(ayt[b], (y0, cols), (1.0 - wy))
        np.add.at(ayt[b], (y1, cols), wy)
        x0, x1, wx = _axis_weights(int(j[b]), int(w[b]), HM)
        if flip_mask[b]:
            x0 = HM - x0
            x1 = HM - x1
        np.add.at(axt[b], (x0, cols), (1.0 - wx))
        np.add.at(axt[b], (x1, cols), wx)
    return ayt, axt


# ---------------------------------------------------------------- device prog
def _build_program(use_meta):
    import concourse.mybir as mybir
    import concourse.tile as tile
    from concourse import bacc

    DT = getattr(mybir.dt, DT_NAME)
    F32 = mybir.dt.float32

    nc = bacc.Bacc(None)
    x_d = nc.dram_tensor("x_s", [BPC, C, HM, WM], DT, kind="ExternalInput")
    ayt_d = nc.dram_tensor("ayt", [BPC, HM, NP], DT, kind="ExternalInput")
    axt_d = nc.dram_tensor("axt", [BPC, WM, NP], DT, kind="ExternalInput")
    out_d = nc.dram_tensor("out", [BPC, C, SIZE, SIZE], F32, kind="ExternalOutput")

    with tile.TileContext(nc) as tc:
        with (
            tc.tile_pool(name="img", bufs=10) as img_pool,
            tc.tile_pool(name="wy", bufs=5) as wy_pool,
            tc.tile_pool(name="wx", bufs=5) as wx_pool,
            tc.tile_pool(name="tq", bufs=24) as tq_pool,
            tc.tile_pool(name="ob", bufs=10) as out_pool,
            tc.tile_pool(name="ps1", bufs=5, space="PSUM") as ps1,
            tc.tile_pool(name="ps2", bufs=3, space="PSUM") as ps2,
        ):

            def emit_stage2(job):
                b, c, tqs, tmeta, axt_t, axt_m = job
                for m2 in range(2):
                    m2sz = min(128, SIZE - m2 * 128)
                    sl = slice(m2 * 128, m2 * 128 + m2sz)
                    po = ps2.tile([128, NP], F32, tag="ps2")
                    for k2 in range(KY):
                        nc.tensor.matmul(
                            po[:m2sz, :],
                            tqs[k2][:, sl],
                            axt_t[:, k2, :],
                            start=(k2 == 0),
                            stop=(k2 == KY - 1 and not use_meta),
                        )
                    if use_meta:
                        nc.tensor.matmul(
                            po[:m2sz, :],
                            tmeta[:1, sl],
                            axt_m[:1, :],
                            start=False,
                            stop=True,
                        )
                    ob = out_pool.tile([128, SIZE], F32, tag="ob")
                    nc.scalar.copy(ob[:m2sz, :], po[:m2sz, :SIZE])
                    nc.sync.dma_start(out_d[b, c, sl, :], ob[:m2sz, :])

            pending = []
            PIPE_DEPTH = 4
            for b in range(BPC):
                # Row-interleaved k-tiles: partition p of k-tile a holds source
                # row 4p+a, so each partition's DMA run is 4 consecutive rows
                # (contiguous in HBM). img below uses the same mapping, so the
                # matmul contraction pairs identical rows — order is irrelevant.
                ayt_t = wy_pool.tile([128, KY, NP], DT, tag="wy")
                nc.sync.dma_start(
                    ayt_t[:], ayt_d[b].rearrange("(p a) n -> p a n", p=128)
                )
                axt_t = wx_pool.tile([128, KY, NP], DT, tag="wx")
                nc.sync.dma_start(
                    axt_t[:], axt_d[b, :HM].rearrange("(a p) n -> p a n", p=128)
                )
                axt_m = None
                if use_meta:
                    axt_m = wx_pool.tile([1, NP], DT, tag="wxm")
                    nc.sync.dma_start(axt_m[:], axt_d[b, HM : HM + 1, :])
                for c in range(C):
                    # Two separate tiles (not one) so stage-1's k=0,1 matmuls
                    # depend only on the first DMA, not both halves.
                    img_lo = img_pool.tile([128, 2, WM], DT, tag="img_lo")
                    img_hi = img_pool.tile([128, 2, WM], DT, tag="img_hi")
                    xv = x_d[b, c].rearrange("(p a) w -> p a w", p=128)
                    nc.sync.dma_start(img_lo[:], xv[:, 0:2, :])
                    nc.sync.dma_start(img_hi[:], xv[:, 2:4, :])

                    def img_k(k):
                        return img_lo[:, k, :] if k < 2 else img_hi[:, k - 2, :]
                    # k-outer: all q accumulation groups open at once so the
                    # k=0,1 matmuls only wait on the first img half-DMA.
                    p1s = [
                        ps1.tile([128, NP], F32, tag="ps1", name=f"p1_{b}_{c}_{q}")
                        for q in range(KY)
                    ]
                    for k in range(KY):
                        for q in range(KY):
                            nc.tensor.matmul(
                                p1s[q][:, :],
                                img_k(k)[:, q * 128 : (q + 1) * 128],
                                ayt_t[:, k, :],
                                start=(k == 0),
                                stop=(k == KY - 1),
                            )
                    tqs = []
                    for q in range(KY):
                        tq = tq_pool.tile([128, SIZE], DT, tag="tq")
                        nc.vector.tensor_copy(tq[:], p1s[q][:, :SIZE])
                        tqs.append(tq)
                    tmeta = None
                    if use_meta:
                        pm = ps1.tile([128, NP], F32, tag="ps1")
                        for k in range(KY):
                            nc.tensor.matmul(
                                pm[:1, :],
                                img_k(k)[:, HM : HM + 1],
                                ayt_t[:, k, :],
                                start=(k == 0),
                                stop=(k == KY - 1),
                            )
                        tmeta = tq_pool.tile([1, SIZE], DT, tag="tqm")
                        nc.vector.tensor_copy(tmeta[:], pm[:1, :SIZE])
                    pending.append((b, c, tqs, tmeta, axt_t, axt_m))
                    if len(pending) > PIPE_DEPTH:
                        emit_stage2(pending.pop(0))
            for job in pending:
                emit_stage2(job)

    nc.compile()
    return nc


def _get_program(use_meta):
    key = (use_meta, DT_NAME, NP)
    if key not in _prog_cache:
        _prog_cache[key] = _build_program(use_meta)
    return _prog_cache[key]


def _np_dt(name):
    if name == "bfloat16":
        import ml_dtypes

        return ml_dtypes.bfloat16
    if name == "float16":
        return np.float16
    return np.float32


def _round_f32r(a):
    """Pre-round fp32 data to the FP32R grid the PE uses, so device inputs
    are already 'rounded to FP32r' and results are deterministic."""
    if F32R_BITS is None or F32R_BITS >= 23:
        return a
    drop = 23 - F32R_BITS
    u = np.ascontiguousarray(a).view(np.uint32)
    if F32R_MODE == "rne":
        half = np.uint32((1 << (drop - 1)) - 1)
        lsb = (u >> drop) & np.uint32(1)
        u = u + half + lsb
    u = (u >> drop) << drop
    return u.view(np.float32)


# ---------------------------------------------------------------- entry point
def kernel(x, trace=False):
    from concourse.bass_utils import run_bass_kernel_spmd

    x = np.ascontiguousarray(np.asarray(x, dtype=np.float32))
    assert x.shape == (B_FULL, C, HM, WM), x.shape

    ayt, axt = _build_weights(x)
    use_meta = bool(np.abs(axt[:, HM, :]).max() > 0)
    import os

    if os.environ.get("KERNEL_FORCE_USE_META"):
        use_meta = True

    nc = _get_program(use_meta)

    ndt = _np_dt(DT_NAME)
    xc = x.astype(ndt, copy=False)
    aytc = ayt.astype(ndt, copy=False)
    axtc = axt.astype(ndt, copy=False)
    if DT_NAME == "float32r":
        xc = _round_f32r(xc)
        aytc = _round_f32r(aytc)
        axtc = _round_f32r(axtc)

    in_maps = []
    for k in range(N_CORES):
        sl = slice(k * BPC, (k + 1) * BPC)
        in_maps.append(
            {
                "x_s": np.ascontiguousarray(xc[sl]),
                "ayt": np.ascontiguousarray(aytc[sl]),
                "axt": np.ascontiguousarray(axtc[sl]),
            }
        )

    res = run_bass_kernel_spmd(nc, in_maps, list(range(N_CORES)), trace=trace)
    out = np.concatenate([res.results[k]["out"] for k in range(N_CORES)], axis=0)
    if trace:
        kernel.last_exec_ns = res.exec_time_ns
        kernel.last_results = res
    return out
